# revision 16
# baseline (speedup 1.0000x reference)
"""Trainium2 Bass kernel: single-head causal attention, SPMD over 8 NeuronCores.

Problem: x [4, 2048, 1024] f32; Wq/Wk/Wv [1024, 64]; bq/bk/bv [64].
  q,k,v = x@W + b ; out = softmax(causal(q k^T / 8)) @ v  -> [4, 2048, 64]

Sharding (uniform SPMD structure on every core):
  core c -> batch b = c//2 ; query chunks (cA, cB) = (c%2, 3-c%2), 512 rows
  each (pairing an early with a late chunk balances causal work).  Every core
  computes K/V for its batch's full 2048 rows.

Key layout trick: the k-axis is permuted PER CORE to chunk order
  [cA, 1-cA, 5-cB, cB], so the core's own query columns sit at STATIC
  positions 0:512 and 1536:2048 of the K/V input -- Q projection needs no
  separate input tensor.  Causality comes from host-precomputed per-core 0/1
  mask tiles (diagonal tiles) and exp-bias kills (fully-masked tiles), which
  absorb the permutation.

  Projections produce Q^T/K^T/V^T [64, rows]; scores are computed transposed
  ([k_part, q_free]) so the attention-weight matrix feeds the AV matmul as
  the moving operand; V is re-transposed through 16 small PE transposes; a
  65th "ones" row on the V tiles makes the AV matmul accumulate the softmax
  denominator for free.  Score matmuls (K=64) are row-packed in pairs into
  disjoint PE row-groups; the partition-64:127 K^T/Q^T duplicates they need
  are produced by partition-shifted bias-adds straight from PSUM (DVE/ACT
  read partitions 0:64, write 64:128) -- cheaper than SBUF->SBUF DMA
  (~0.65us sequencer issue + ~2us completion each).

Schedule: the hard resource is the ACT engine (24 exp tiles x ~0.58us,
  1 elem/cycle/lane regardless of dtype).  x streams K-MAJOR -- four
  1MB blocks of k-positions (K0=0:512, K1=512:1024, K3=1536:2048,
  K2=1024:1536), each holding all 8 d-tiles for those positions -- so the
  first K/V block (and the first scores-exp) is ready after ~1MB of
  stream + 16 matmuls instead of after the whole 2.5MB h0 half.  The exp
  chain then runs near-gapless while the remaining blocks stream/project.
  Inputs are FEW LARGE DMAs (one dma_start spans all 16 SDMA engines at
  ~340GB/s; issue costs ~0.65us sequencer time each, so count is key) in
  priority order on the sync queue.  Slot-A attention interleaves with the
  later projections on the PE (PSUM: proj kv 1 + q 2 + score 3 + av 2 = 8
  banks; the 2-bank av pool is reused A->B).  Slot-A merge runs on DVE
  (ACT is mid-exp); slot-B merge on ACT (free after the last exp).

dtypes: fp16 SBUF operands, fp32 PSUM accumulation, fp32 normalize.
"""

import os
import sys

import numpy as np

if "/opt/trn_rl_repo" not in sys.path:
    sys.path.insert(0, "/opt/trn_rl_repo")

B, S, D, H = 4, 2048, 1024, 64
CH = 512          # query chunk width
QR = 2 * CH       # query rows per core
NKT = S // 128    # 16 k-tiles of 128
SCALE = 1.0 / np.sqrt(H)

# fp16 blob column layout -- weights/identities at the head
C_WKV = 0          # [128, 1024] 8 d blocks of [128, 128]
C_WQ = 1024        # [128, 512]  8 d blocks of [128, 64]
C_IDV = 1536       # [128, 64]   eye(64) stacked twice
C_ID16 = 1600      # [65, 65]    eye(65)
C16_N = 1696       # padded total
# blob chunk map (name, col0, col1): DMA/stream order
BLOB_CH = [("hd", 0, C16_N + 2048),               # cst16 + K0 d0..3
           ("k0b", C16_N + 2048, C16_N + 4096),   # K0 d4..7
           ("k1", C16_N + 4096, C16_N + 8192),
           ("k3", C16_N + 8192, C16_N + 12288),
           ("msk", C16_N + 12288, C16_N + 16384),
           ("k2", C16_N + 16384, C16_N + 20480)]
BLOB_N = C16_N + 20480

# cst32 column layout (f32)
C_BKV = 0          # [128, 1]
C_BQ = 1           # [64, 1]
C_THRB = 2         # [128, 32] exp bias: 0 or -1e5 (fully-masked kill)
C32_N = 34

# xk k-major block layout: block bi covers k-positions KPOS[bi]:+512 with
# 8 d-tiles of [128, 512] each; stream order K0, K1, K3, K2.
KPOS = (0, 512, 1536, 1024)
# kvT 512-col block index (nb) per stream block
KNB = (0, 1, 3, 2)

_CACHE = {}


def _build_nc():
    import concourse.bacc as bacc
    import concourse.mybir as mybir
    import concourse.tile as tile

    DT = mybir.dt.float16
    F32 = mybir.dt.float32
    Exp = mybir.ActivationFunctionType.Exp
    Copy = mybir.ActivationFunctionType.Copy
    Ident = mybir.ActivationFunctionType.Identity
    mult = mybir.AluOpType.mult
    add = mybir.AluOpType.add

    nc = bacc.Bacc("TRN2", target_bir_lowering=False, debug=False, num_devices=8)

    blob = nc.dram_tensor("blob", [128, BLOB_N], DT, kind="ExternalInput")
    cst32 = nc.dram_tensor("cst32", [128, C32_N], F32, kind="ExternalInput")
    out = nc.dram_tensor("out", [128, 8 * H], DT, kind="ExternalOutput")

    with tile.TileContext(nc) as tc:
        with (
            tc.tile_pool(name="const", bufs=1) as cp,
            tc.tile_pool(name="work", bufs=16) as wp,
            tc.tile_pool(name="epi", bufs=4) as ep,
        ):
            # ---- head: 6 large DMAs in priority order on the sync
            # queue; weights+K0a share ONE DMA (each extra dma_start on the
            # critical path costs ~1us issue+drain-start lag)
            tiles = {}
            for nm, c0, c1 in BLOB_CH:
                tiles[nm] = cp.tile([128, c1 - c0], DT, tag=nm, name=nm)
                nc.sync.dma_start(tiles[nm][:], blob[:, c0:c1])
            hd_sb, xb0b = tiles["hd"], tiles["k0b"]
            xb1, xb3, xb2 = tiles["k1"], tiles["k3"], tiles["k2"]
            msk_sb = tiles["msk"]
            cst32_sb = cp.tile([128, C32_N], F32, tag="cst32", name="cst32")
            nc.gpsimd.dma_start(cst32_sb[:], cst32[:])

            wkv_sb = hd_sb[:, C_WKV:C_WKV + 1024]
            wq_sb = hd_sb[:, C_WQ:C_WQ + 512]
            idv_sb = hd_sb[:, C_IDV:C_IDV + H]
            id16_sb = hd_sb[0:H + 1, C_ID16:C_ID16 + H + 1]
            bkv_sb = cst32_sb[:, C_BKV:C_BKV + 1]
            bq_sb = cst32_sb[0:H, C_BQ:C_BQ + 1]
            thrb_sb = cst32_sb[:, C_THRB:C_THRB + 2 * NKT]

            def xs(bi, d):    # stream block bi, d-tile d -> [128, 512]
                if bi == 0:
                    if d < 4:
                        return hd_sb[:, C16_N + d * 512:C16_N + (d + 1) * 512]
                    return xb0b[:, (d - 4) * 512:(d - 3) * 512]
                t = (xb1, xb3, xb2)[bi - 1]
                return t[:, d * 512:(d + 1) * 512]

            kvT_sb = cp.tile([128, S], DT, tag="kvT", name="kvT")  # 0:64 K^T, 64:128 V^T
            qT_sb = cp.tile([H, QR], DT, tag="qT", name="qT")      # A cols 0:512, B 512:1024
            v_sb = cp.tile([128, NKT * (H + 1)], DT, tag="v", name="v")
            # duplicates at partitions 64:127 for row-packed score pairs
            ktd_sb = cp.tile([128, S], DT, tag="ktd", name="ktd")
            qTd_sb = cp.tile([128, QR], DT, tag="qTd", name="qTd")
            vtd_sb = cp.tile([64, S], DT, tag="vtd", name="vtd")
            o_all = cp.tile([128, 8 * H], DT, tag="oall", name="oall")

            nc.vector.memset(v_sb[:], 1.0)

            pk = tc.alloc_tile_pool(name="proj_ps", bufs=1, space="PSUM")
            pq = tc.alloc_tile_pool(name="q_ps", bufs=1, space="PSUM")
            sp = tc.alloc_tile_pool(name="score_ps", bufs=4, space="PSUM")
            avp = tc.alloc_tile_pool(name="av_ps", bufs=1, space="PSUM")

            def kv_block(bi):
                """8 accumulating matmuls for one 512-position k block, then
                DVE epilogue (ktd dup first: it unblocks row-packed scores)."""
                kv_ps = pk.tile([128, 512], F32, tag="kvps", name="kvps")
                for d in range(8):
                    nc.tensor.matmul(
                        kv_ps[:], wkv_sb[:, d * 128:(d + 1) * 128],
                        xs(bi, d), start=(d == 0), stop=(d == 7))
                nb = KNB[bi]
                cs = slice(nb * 512, (nb + 1) * 512)
                nc.vector.tensor_scalar(ktd_sb[H:128, cs], kv_ps[0:H, :],
                                        bkv_sb[0:H, :], None, add)
                nc.vector.tensor_scalar(kvT_sb[:, cs], kv_ps[:],
                                        bkv_sb[:], None, add)
                nc.vector.tensor_scalar(vtd_sb[:, cs], kv_ps[H:128, :],
                                        bkv_sb[H:128, :], None, add)

            def q_block(bi, slot):
                q_ps = pq.tile([H, 512], F32, tag="qps", name="qps")
                for d in range(8):
                    nc.tensor.matmul(
                        q_ps[:], wq_sb[:, d * H:(d + 1) * H],
                        xs(bi, d), start=(d == 0), stop=(d == 7))
                cs = slice(slot * 512, (slot + 1) * 512)
                if slot == 0:     # ACT is idle pre-exp; keep DVE for kv epis
                    nc.scalar.activation(qT_sb[:, cs], q_ps[:], Ident,
                                         bias=bq_sb[:])
                    nc.scalar.activation(qTd_sb[H:128, cs], q_ps[:], Ident,
                                         bias=bq_sb[:])
                else:             # ACT is mid-exp by now
                    nc.vector.tensor_scalar(qT_sb[:, cs], q_ps[:],
                                            bq_sb[:], None, add)
                    nc.vector.tensor_scalar(qTd_sb[H:128, cs], q_ps[:],
                                            bq_sb[:], None, add)

            def v_transpose(pr):
                k0, k1 = 2 * pr, 2 * pr + 1
                t0 = sp.tile([128, H], DT, tag="score", name="vtr0")
                nc.tensor.transpose(
                    t0[:], vtd_sb[:, k0 * 128:(k0 + 1) * 128],
                    idv_sb[0:H, :], tile_position=(0, 0))
                t1 = sp.tile([128, H], DT, tag="score", name="vtr1")
                nc.tensor.transpose(
                    t1[:], kvT_sb[64:128, k1 * 128:(k1 + 1) * 128],
                    idv_sb[64:64 + H, :], tile_position=(64, 0))
                nc.vector.tensor_copy(
                    v_sb[:, k0 * (H + 1):k0 * (H + 1) + H], t0[:])
                nc.vector.tensor_copy(
                    v_sb[:, k1 * (H + 1):k1 * (H + 1) + H], t1[:])

            def score_pair(slot, kt0, kt1):
                s0 = sp.tile([128, 512], F32, tag="score", name="score0")
                nc.tensor.matmul(
                    s0[:], kvT_sb[0:H, kt0 * 128:(kt0 + 1) * 128],
                    qT_sb[:, slot * 512:(slot + 1) * 512],
                    start=True, stop=True, tile_position=(0, 0))
                s1 = sp.tile([128, 512], F32, tag="score", name="score1")
                nc.tensor.matmul(
                    s1[:], ktd_sb[H:128, kt1 * 128:(kt1 + 1) * 128],
                    qTd_sb[H:128, slot * 512:(slot + 1) * 512],
                    start=True, stop=True, tile_position=(64, 0))
                return s0, s1

            def mask_mult(kt, slot, w_sb):
                m = kt if slot == 0 else kt - 8
                wm_sb = wp.tile([128, 512], DT, tag="wm", name="wm")
                nc.vector.tensor_tensor(
                    wm_sb[:], w_sb[:], msk_sb[:, m * 512:(m + 1) * 512], mult)
                return wm_sb

            def exp_pair(slot, kt0, kt1, s0, s1, defer_mult=False):
                """exps on ACT; diag tiles get a DVE mask multiply, which can
                be deferred so exp-gated mults don't clog the DVE queue
                ahead of spine-critical kv epilogues."""
                res = []
                for kt, s_ps in zip((kt0, kt1), (s0, s1)):
                    idx = slot * NKT + kt
                    w_sb = wp.tile([128, 512], DT, tag="wexp", name="wexp")
                    nc.scalar.activation(w_sb[:], s_ps[:], Exp,
                                         bias=thrb_sb[:, idx:idx + 1],
                                         scale=float(SCALE))
                    diag = (slot == 0 and kt < 4) or (slot == 1 and kt >= 12)
                    if diag and not defer_mult:
                        res.append(mask_mult(kt, slot, w_sb))
                    else:
                        res.append(w_sb)
                return res

            def av_accum(av_e, av_o, kt, w_av, first, last):
                vs = slice(kt * (H + 1), (kt + 1) * (H + 1))
                nc.tensor.matmul(
                    av_e[:], v_sb[0:H, vs], w_av[0:H, :],
                    start=first, stop=last, tile_position=(0, 0))
                nc.tensor.matmul(
                    av_o[:], v_sb[H:128, vs], w_av[H:128, :],
                    start=first, stop=last, tile_position=(64, 0))

            # ================= emission (PE-queue order) =================
            kv_block(0)                       # K0 -> kvT 0:512
            q_block(0, 0)                     # qA (+ ACT idents)
            kv_block(1)                       # K1 -> kvT 512:1024
            # (kv1's DVE epilogue queues BEFORE slot-A mults/v-copies so
            # the kt4..7 scores are never DVE-starved)

            avA_e = avp.tile([H + 1, 512], F32, tag="avE", name="avE")
            avA_o = avp.tile([H + 1, 512], F32, tag="avO", name="avO")
            sA = [score_pair(0, 2 * p, 2 * p + 1) for p in range(2)]
            wA = [exp_pair(0, 2 * p, 2 * p + 1, *sA[p], defer_mult=True)
                  for p in range(2)]

            q_block(2, 1)                     # qB early: its DVE epilogue
            # (qTd-B) gates the slot-B half of the exp spine
            for p in range(2):
                for j in range(2):
                    wA[p][j] = mask_mult(2 * p + j, 0, wA[p][j])

            sA2 = [score_pair(0, 4 + 2 * p, 5 + 2 * p) for p in range(2)]
            wA2 = [exp_pair(0, 4 + 2 * p, 5 + 2 * p, *sA2[p]) for p in range(2)]
            kts = list(range(8)) + [12, 13, 14, 15, 8, 9, 10, 11]
            wB = {}
            for p in range(4):                # slot B scores kt 0..7, ahead
                kt0, kt1 = kts[2 * p], kts[2 * p + 1]   # of the V transposes
                s0, s1 = score_pair(1, kt0, kt1)        # so ACT never waits
                wB[p] = exp_pair(1, kt0, kt1, s0, s1)
            for pr in range(4):
                v_transpose(pr)

            for p in range(2):                # slot A AV kt 0..3
                for j in range(2):
                    av_accum(avA_e, avA_o, 2 * p + j, wA[p][j],
                             2 * p + j == 0, False)

            kv_block(2)                       # K3 -> kvT 1536:2048
            for p in range(2):                # slot A AV kt 4..7 (covers the
                for j in range(2):            # kv_ps WAR window K3->K2)
                    av_accum(avA_e, avA_o, 4 + 2 * p + j, wA2[p][j],
                             False, 4 + 2 * p + j == 7)
            kv_block(3)                       # K2 -> kvT 1024:1536
            for p in (4, 5):                  # slot B scores kt 12..15
                kt0, kt1 = kts[2 * p], kts[2 * p + 1]
                s0, s1 = score_pair(1, kt0, kt1)
                wB[p] = exp_pair(1, kt0, kt1, s0, s1, defer_mult=True)
            for pr in (6, 7):
                v_transpose(pr)

            for p in (6, 7):                  # slot B scores kt 8..11
                kt0, kt1 = kts[2 * p], kts[2 * p + 1]
                s0, s1 = score_pair(1, kt0, kt1)
                wB[p] = exp_pair(1, kt0, kt1, s0, s1)
            for pr in (4, 5):
                v_transpose(pr)

            # slot A merge on DVE (ACT mid-exp); frees av pool for slot B
            oavA = ep.tile([H + 1, 512], DT, tag="oavA", name="oavA")
            ocA = ep.tile([H + 1, 512], F32, tag="ocA", name="ocA")
            for j in range(4):
                js = slice(j * 128, (j + 1) * 128)
                nc.vector.tensor_copy(ocA[:, js], avA_e[:, js])
                nc.vector.tensor_tensor(oavA[:, js], ocA[:, js],
                                        avA_o[:, js], add)
            # deferred slot-B diagonal mask mults (kt 12..15)
            for p in (4, 5):
                for j in range(2):
                    wB[p][j] = mask_mult(kts[2 * p + j], 1, wB[p][j])

            avB_e = avp.tile([H + 1, 512], F32, tag="avE", name="avE")
            avB_o = avp.tile([H + 1, 512], F32, tag="avO", name="avO")
            for p in range(4):                # slot B AV kt 0..7
                for j in range(2):
                    i = 2 * p + j
                    av_accum(avB_e, avB_o, kts[i], wB[p][j], i == 0, False)

            # slot A transpose + normalize + store
            for j in range(4):
                tr_ps = sp.tile([128, H + 1], DT, tag="score", name="otrA")
                nc.tensor.transpose(tr_ps[:], oavA[:, j * 128:(j + 1) * 128],
                                    id16_sb[:])
                r_sb = ep.tile([128, 1], F32, tag="recip", name="recip")
                nc.vector.reciprocal(r_sb[:], tr_ps[:, H:H + 1])
                o_col = j * H
                nc.vector.tensor_scalar_mul(
                    o_all[:, o_col:o_col + H], tr_ps[:, 0:H], r_sb[:])
            nc.sync.dma_start(out[:, 0:4 * H], o_all[:, 0:4 * H])

            for p in (4, 5, 6, 7):            # slot B AV kt 12..15, 8..11
                for j in range(2):
                    i = 2 * p + j
                    av_accum(avB_e, avB_o, kts[i], wB[p][j], False, i == 15)

            # slot B tail: merge on ACT (free after exps) + DVE
            oavB = ep.tile([H + 1, 512], DT, tag="oavB", name="oavB")
            ocB = ep.tile([H + 1, 512], F32, tag="ocB", name="ocB")
            for j in range(4):
                js = slice(j * 128, (j + 1) * 128)
                nc.scalar.activation(ocB[:, js], avB_e[:, js], Copy)
                nc.vector.tensor_tensor(oavB[:, js], ocB[:, js],
                                        avB_o[:, js], add)
                tr_ps = sp.tile([128, H + 1], DT, tag="score", name="otrB")
                nc.tensor.transpose(tr_ps[:], oavB[:, js], id16_sb[:])
                r_sb = ep.tile([128, 1], F32, tag="recip", name="recip")
                nc.vector.reciprocal(r_sb[:], tr_ps[:, H:H + 1])
                o_col = (4 + j) * H
                nc.vector.tensor_scalar_mul(
                    o_all[:, o_col:o_col + H], tr_ps[:, 0:H], r_sb[:])
            nc.scalar.dma_start(out[:, 4 * H:8 * H], o_all[:, 4 * H:8 * H])

            for pool in (avp, sp, pq, pk):
                pool.release()

    nc.compile()
    return nc


def _host_inputs(x, Wq, bq, Wk, bk, Wv, bv):
    """Build the 8 per-core input maps (all SBUF-layout, fp16/f32)."""
    f16 = np.float16
    Wkv = np.concatenate([Wk, Wv], axis=1)          # [D, 128]

    cst16_np = np.zeros((128, C16_N), dtype=f16)
    for d in range(8):
        cst16_np[:, C_WKV + d * 128:C_WKV + (d + 1) * 128] = \
            Wkv[d * 128:(d + 1) * 128, :]
        cst16_np[:, C_WQ + d * H:C_WQ + (d + 1) * H] = \
            Wq[d * 128:(d + 1) * 128, :]
    cst16_np[:, C_IDV:C_IDV + H] = np.concatenate(
        [np.eye(H), np.eye(H)], axis=0)
    cst16_np[0:H + 1, C_ID16:C_ID16 + H + 1] = np.eye(H + 1)
    xoff = {0: C16_N, 1: C16_N + 4096, 2: C16_N + 8192, 3: C16_N + 16384}

    in_maps = []
    for c in range(8):
        b = c // 2
        cA, cB = c % 2, 3 - c % 2
        perm = (cA, 1 - cA, 5 - cB, cB)        # chunk order along k
        xTp = np.concatenate(
            [x[b, p * CH:(p + 1) * CH].T for p in perm], axis=1)  # [D, S]
        xTp = xTp.astype(f16)
        blob_np = np.zeros((128, BLOB_N), dtype=f16)
        blob_np[:, 0:C16_N] = cst16_np
        for bi in range(4):
            kp = KPOS[bi]
            for d in range(8):
                o = xoff[bi] + d * 512
                blob_np[:, o:o + 512] = \
                    xTp[d * 128:(d + 1) * 128, kp:kp + 512]
        # k_global of permuted position p: perm[p//512]*512 + p%512
        pos = np.arange(S)
        kg = np.array(perm)[pos // CH] * CH + pos % CH
        thr_np = np.zeros((128, 2 * NKT), dtype=np.float32)
        p = np.arange(128)
        for slot, ck in enumerate((cA, cB)):
            for kt in range(NKT):
                thr_np[:, slot * NKT + kt] = kg[kt * 128 + p] - ck * CH
        thrb_np = np.zeros((128, 2 * NKT), dtype=np.float32)
        for slot in range(2):
            for kt in range(NKT):
                diag = (slot == 0 and kt < 4) or (slot == 1 and kt >= 12)
                if diag:
                    continue
                col = thr_np[:, slot * NKT + kt]
                if np.all(col <= 0):
                    continue          # fully visible -> bias 0
                thrb_np[:, slot * NKT + kt] = -1e5   # fully masked
        qio = np.arange(CH, dtype=np.float32)[None, :]
        for m in range(8):
            idx = m if m < 4 else NKT + 8 + m
            o = C16_N + 12288 + m * 512
            blob_np[:, o:o + 512] = \
                (qio >= thr_np[:, idx:idx + 1]).astype(f16)
        cst32_np = np.zeros((128, C32_N), dtype=np.float32)
        cst32_np[:, C_BKV] = np.concatenate([bk, bv])
        cst32_np[0:H, C_BQ] = bq
        cst32_np[:, C_THRB:C_THRB + 2 * NKT] = thrb_np
        in_maps.append({"blob": blob_np, "cst32": cst32_np})
    return in_maps


def _gather(results, dtype):
    y = np.zeros((B, S, H), dtype=dtype)
    for c in range(8):
        b = c // 2
        cA, cB = c % 2, 3 - c % 2
        o = results[c]["out"]
        for slot, ck in enumerate((cA, cB)):
            for j in range(4):
                col = (slot * 4 + j) * H
                y[b, ck * CH + j * 128:ck * CH + (j + 1) * 128] = \
                    o[:, col:col + H]
    return y


def get_nc():
    if "nc" not in _CACHE:
        _CACHE["nc"] = _build_nc()
    return _CACHE["nc"]


def kernel(x, Wq, bq, Wk, bk, Wv, bv, _trace=False, _trace_kwargs=None):
    from concourse.bass_utils import run_bass_kernel_spmd

    x = np.asarray(x, dtype=np.float32)
    Wq, bq = np.asarray(Wq, np.float32), np.asarray(bq, np.float32)
    Wk, bk = np.asarray(Wk, np.float32), np.asarray(bk, np.float32)
    Wv, bv = np.asarray(Wv, np.float32), np.asarray(bv, np.float32)

    nc = get_nc()
    in_maps = _host_inputs(x, Wq, bq, Wk, bk, Wv, bv)
    res = run_bass_kernel_spmd(
        nc, in_maps, core_ids=list(range(8)),
        trace=_trace, **(_trace_kwargs or {}))
    _CACHE["last_result"] = res
    return _gather(res.results, x.dtype)


# revision 17
# speedup vs baseline: 1.0984x; 1.0984x over previous
"""Trainium2 Bass kernel: single-head causal attention, SPMD over 8 NeuronCores.

Problem: x [4, 2048, 1024] f32; Wq/Wk/Wv [1024, 64]; bq/bk/bv [64].
  q,k,v = x@W + b ; out = softmax(causal(q k^T / 8)) @ v  -> [4, 2048, 64]

Sharding (uniform SPMD structure on every core):
  core c -> batch b = c//2 ; query chunks (cA, cB) = (c%2, 3-c%2), 512 rows
  each (pairing an early with a late chunk balances causal work).  Every core
  computes K/V for its batch's full 2048 rows.

Key layout trick: the k-axis is permuted PER CORE to chunk order
  [cA, 1-cA, 5-cB, cB], so the core's own query columns sit at STATIC
  positions 0:512 and 1536:2048 of the K/V input -- Q projection needs no
  separate input tensor.  Causality comes from host-precomputed per-core 0/1
  mask tiles (diagonal tiles) and exp-bias kills (fully-masked tiles), which
  absorb the permutation.

  Projections produce Q^T/K^T/V^T [64, rows]; scores are computed transposed
  ([k_part, q_free]) so the attention-weight matrix feeds the AV matmul as
  the moving operand; V is re-transposed through 16 small PE transposes; a
  65th "ones" row on the V tiles makes the AV matmul accumulate the softmax
  denominator for free.  Score matmuls (K=64) are row-packed in pairs into
  disjoint PE row-groups; the partition-64:127 K^T/Q^T duplicates they need
  are produced by partition-shifted bias-adds straight from PSUM (DVE/ACT
  read partitions 0:64, write 64:128) -- cheaper than SBUF->SBUF DMA
  (~0.65us sequencer issue + ~2us completion each).

Schedule: the hard resource is the ACT engine (24 exp tiles x ~0.58us,
  1 elem/cycle/lane regardless of dtype).  x streams K-MAJOR -- four
  1MB blocks of k-positions (K0=0:512, K1=512:1024, K3=1536:2048,
  K2=1024:1536), each holding all 8 d-tiles for those positions -- so the
  first K/V block (and the first scores-exp) is ready after ~1MB of
  stream + 16 matmuls instead of after the whole 2.5MB h0 half.  The exp
  chain then runs near-gapless while the remaining blocks stream/project.
  Inputs are FEW LARGE DMAs (one dma_start spans all 16 SDMA engines at
  ~340GB/s; issue costs ~0.65us sequencer time each, so count is key) in
  priority order on the sync queue.  Slot-A attention interleaves with the
  later projections on the PE (PSUM: proj kv 1 + q 2 + score 3 + av 2 = 8
  banks; the 2-bank av pool is reused A->B).  Slot-A merge runs on DVE
  (ACT is mid-exp); slot-B merge on ACT (free after the last exp).

dtypes: fp16 SBUF operands, fp32 PSUM accumulation, fp32 normalize.
"""

import os
import sys

import numpy as np

if "/opt/trn_rl_repo" not in sys.path:
    sys.path.insert(0, "/opt/trn_rl_repo")

B, S, D, H = 4, 2048, 1024, 64
CH = 512          # query chunk width
QR = 2 * CH       # query rows per core
NKT = S // 128    # 16 k-tiles of 128
SCALE = 1.0 / np.sqrt(H)

# fp16 blob column layout -- weights/identities at the head
C_WKV = 0          # [128, 1024] 8 d blocks of [128, 128]
C_WQ = 1024        # [128, 512]  8 d blocks of [128, 64]
C_IDV = 1536       # [128, 64]   eye(64) stacked twice
C_ID16 = 1600      # [65, 65]    eye(65)
C16_N = 1696       # padded total
# blob chunk map (name, col0, col1): DMA/stream order
BLOB_CH = [("hd", 0, C16_N + 2048),               # cst16 + K0 d0..3
           ("k0b", C16_N + 2048, C16_N + 4096),   # K0 d4..7
           ("k1", C16_N + 4096, C16_N + 8192),
           ("k3", C16_N + 8192, C16_N + 12288),
           ("msk", C16_N + 12288, C16_N + 16384),
           ("k2", C16_N + 16384, C16_N + 20480)]
BLOB_N = C16_N + 20480

# cst32 column layout (f32)
C_BKV = 0          # [128, 1]
C_BQ = 1           # [64, 1]
C_THRB = 2         # [128, 32] exp bias: 0 or -1e5 (fully-masked kill)
C32_N = 34

# xk k-major block layout: block bi covers k-positions KPOS[bi]:+512 with
# 8 d-tiles of [128, 512] each; stream order K0, K1, K3, K2.
KPOS = (0, 512, 1536, 1024)
# kvT 512-col block index (nb) per stream block
KNB = (0, 1, 3, 2)

_CACHE = {}


def _build_nc():
    import concourse.bacc as bacc
    import concourse.mybir as mybir
    import concourse.tile as tile

    DT = mybir.dt.float16
    F32 = mybir.dt.float32
    Exp = mybir.ActivationFunctionType.Exp
    Copy = mybir.ActivationFunctionType.Copy
    Ident = mybir.ActivationFunctionType.Identity
    mult = mybir.AluOpType.mult
    add = mybir.AluOpType.add

    nc = bacc.Bacc("TRN2", target_bir_lowering=False, debug=False, num_devices=8)

    blob = nc.dram_tensor("blob", [128, BLOB_N], DT, kind="ExternalInput")
    cst32 = nc.dram_tensor("cst32", [128, C32_N], F32, kind="ExternalInput")
    out = nc.dram_tensor("out", [128, 8 * H], DT, kind="ExternalOutput")

    with tile.TileContext(nc) as tc:
        with (
            tc.tile_pool(name="const", bufs=1) as cp,
            tc.tile_pool(name="work", bufs=16) as wp,
            tc.tile_pool(name="epi", bufs=4) as ep,
        ):
            # ---- head: 6 large DMAs in priority order on the sync
            # queue; weights+K0a share ONE DMA (each extra dma_start on the
            # critical path costs ~1us issue+drain-start lag)
            tiles = {}
            for nm, c0, c1 in BLOB_CH:
                tiles[nm] = cp.tile([128, c1 - c0], DT, tag=nm, name=nm)
                nc.sync.dma_start(tiles[nm][:], blob[:, c0:c1])
            hd_sb, xb0b = tiles["hd"], tiles["k0b"]
            xb1, xb3, xb2 = tiles["k1"], tiles["k3"], tiles["k2"]
            msk_sb = tiles["msk"]
            cst32_sb = cp.tile([128, C32_N], F32, tag="cst32", name="cst32")
            nc.gpsimd.dma_start(cst32_sb[:], cst32[:])

            wkv_sb = hd_sb[:, C_WKV:C_WKV + 1024]
            wq_sb = hd_sb[:, C_WQ:C_WQ + 512]
            idv_sb = hd_sb[:, C_IDV:C_IDV + H]
            id16_sb = hd_sb[0:H + 1, C_ID16:C_ID16 + H + 1]
            bkv_sb = cst32_sb[:, C_BKV:C_BKV + 1]
            bq_sb = cst32_sb[0:H, C_BQ:C_BQ + 1]
            thrb_sb = cst32_sb[:, C_THRB:C_THRB + 2 * NKT]

            def xs(bi, d):    # stream block bi, d-tile d -> [128, 512]
                if bi == 0:
                    if d < 4:
                        return hd_sb[:, C16_N + d * 512:C16_N + (d + 1) * 512]
                    return xb0b[:, (d - 4) * 512:(d - 3) * 512]
                t = (xb1, xb3, xb2)[bi - 1]
                return t[:, d * 512:(d + 1) * 512]

            kvT_sb = cp.tile([128, S], DT, tag="kvT", name="kvT")  # 0:64 K^T, 64:128 V^T
            qT_sb = cp.tile([H, QR], DT, tag="qT", name="qT")      # A cols 0:512, B 512:1024
            v_sb = cp.tile([128, NKT * (H + 1)], DT, tag="v", name="v")
            # duplicates at partitions 64:127 for row-packed score pairs
            ktd_sb = cp.tile([128, S], DT, tag="ktd", name="ktd")
            qTd_sb = cp.tile([128, QR], DT, tag="qTd", name="qTd")
            vtd_sb = cp.tile([64, S], DT, tag="vtd", name="vtd")
            o_all = cp.tile([128, 8 * H], DT, tag="oall", name="oall")

            nc.vector.memset(v_sb[:], 1.0)

            pk = tc.alloc_tile_pool(name="proj_ps", bufs=1, space="PSUM")
            pq = tc.alloc_tile_pool(name="q_ps", bufs=1, space="PSUM")
            sp = tc.alloc_tile_pool(name="score_ps", bufs=4, space="PSUM")
            avp = tc.alloc_tile_pool(name="av_ps", bufs=1, space="PSUM")

            def kv_block(bi):
                """8 accumulating matmuls for one 512-position k block, then
                DVE epilogue (ktd dup first: it unblocks row-packed scores)."""
                kv_ps = pk.tile([128, 512], F32, tag="kvps", name="kvps")
                for d in range(8):
                    nc.tensor.matmul(
                        kv_ps[:], wkv_sb[:, d * 128:(d + 1) * 128],
                        xs(bi, d), start=(d == 0), stop=(d == 7))
                nb = KNB[bi]
                cs = slice(nb * 512, (nb + 1) * 512)
                nc.vector.tensor_scalar(ktd_sb[H:128, cs], kv_ps[0:H, :],
                                        bkv_sb[0:H, :], None, add)
                nc.vector.tensor_scalar(kvT_sb[:, cs], kv_ps[:],
                                        bkv_sb[:], None, add)
                nc.vector.tensor_scalar(vtd_sb[:, cs], kv_ps[H:128, :],
                                        bkv_sb[H:128, :], None, add)

            def q_block(bi, slot):
                q_ps = pq.tile([H, 512], F32, tag="qps", name="qps")
                for d in range(8):
                    nc.tensor.matmul(
                        q_ps[:], wq_sb[:, d * H:(d + 1) * H],
                        xs(bi, d), start=(d == 0), stop=(d == 7))
                cs = slice(slot * 512, (slot + 1) * 512)
                if slot == 0:     # ACT is idle pre-exp; keep DVE for kv epis
                    nc.scalar.activation(qT_sb[:, cs], q_ps[:], Ident,
                                         bias=bq_sb[:])
                    nc.scalar.activation(qTd_sb[H:128, cs], q_ps[:], Ident,
                                         bias=bq_sb[:])
                else:             # ACT is mid-exp by now
                    nc.vector.tensor_scalar(qT_sb[:, cs], q_ps[:],
                                            bq_sb[:], None, add)
                    nc.vector.tensor_scalar(qTd_sb[H:128, cs], q_ps[:],
                                            bq_sb[:], None, add)

            def v_transpose(pr):
                k0, k1 = 2 * pr, 2 * pr + 1
                t0 = sp.tile([128, H], DT, tag="score", name="vtr0")
                nc.tensor.transpose(
                    t0[:], vtd_sb[:, k0 * 128:(k0 + 1) * 128],
                    idv_sb[0:H, :], tile_position=(0, 0))
                t1 = sp.tile([128, H], DT, tag="score", name="vtr1")
                nc.tensor.transpose(
                    t1[:], kvT_sb[64:128, k1 * 128:(k1 + 1) * 128],
                    idv_sb[64:64 + H, :], tile_position=(64, 0))
                nc.vector.tensor_copy(
                    v_sb[:, k0 * (H + 1):k0 * (H + 1) + H], t0[:])
                nc.vector.tensor_copy(
                    v_sb[:, k1 * (H + 1):k1 * (H + 1) + H], t1[:])

            def score_pair(slot, kt0, kt1):
                s0 = sp.tile([128, 512], F32, tag="score", name="score0")
                nc.tensor.matmul(
                    s0[:], kvT_sb[0:H, kt0 * 128:(kt0 + 1) * 128],
                    qT_sb[:, slot * 512:(slot + 1) * 512],
                    start=True, stop=True, tile_position=(0, 0))
                s1 = sp.tile([128, 512], F32, tag="score", name="score1")
                nc.tensor.matmul(
                    s1[:], ktd_sb[H:128, kt1 * 128:(kt1 + 1) * 128],
                    qTd_sb[H:128, slot * 512:(slot + 1) * 512],
                    start=True, stop=True, tile_position=(64, 0))
                return s0, s1

            def mask_mult(kt, slot, w_sb):
                m = kt if slot == 0 else kt - 8
                wm_sb = wp.tile([128, 512], DT, tag="wm", name="wm")
                nc.vector.tensor_tensor(
                    wm_sb[:], w_sb[:], msk_sb[:, m * 512:(m + 1) * 512], mult)
                return wm_sb

            def exp_pair(slot, kt0, kt1, s0, s1, defer_mult=False):
                """exps on ACT; diag tiles get a DVE mask multiply, which can
                be deferred so exp-gated mults don't clog the DVE queue
                ahead of spine-critical kv epilogues."""
                res = []
                for kt, s_ps in zip((kt0, kt1), (s0, s1)):
                    idx = slot * NKT + kt
                    w_sb = wp.tile([128, 512], DT, tag="wexp", name="wexp")
                    nc.scalar.activation(w_sb[:], s_ps[:], Exp,
                                         bias=thrb_sb[:, idx:idx + 1],
                                         scale=float(SCALE))
                    diag = (slot == 0 and kt < 4) or (slot == 1 and kt >= 12)
                    if diag and not defer_mult:
                        res.append(mask_mult(kt, slot, w_sb))
                    else:
                        res.append(w_sb)
                return res

            def av_accum(av_e, av_o, kt, w_av, first, last):
                vs = slice(kt * (H + 1), (kt + 1) * (H + 1))
                nc.tensor.matmul(
                    av_e[:], v_sb[0:H, vs], w_av[0:H, :],
                    start=first, stop=last, tile_position=(0, 0))
                nc.tensor.matmul(
                    av_o[:], v_sb[H:128, vs], w_av[H:128, :],
                    start=first, stop=last, tile_position=(64, 0))

            # ================= emission (PE-queue order) =================
            kv_block(0)                       # K0 -> kvT 0:512
            q_block(0, 0)                     # qA (+ ACT idents)
            kv_block(1)                       # K1 -> kvT 512:1024
            # (kv1's DVE epilogue queues BEFORE slot-A mults/v-copies so
            # the kt4..7 scores are never DVE-starved)

            avA_e = avp.tile([H + 1, 512], F32, tag="avE", name="avE")
            avA_o = avp.tile([H + 1, 512], F32, tag="avO", name="avO")
            sA = [score_pair(0, 2 * p, 2 * p + 1) for p in range(2)]
            wA = [exp_pair(0, 2 * p, 2 * p + 1, *sA[p], defer_mult=True)
                  for p in range(2)]

            q_block(2, 1)                     # qB early: its DVE epilogue
            # (qTd-B) gates the slot-B half of the exp spine
            for p in range(2):
                for j in range(2):
                    wA[p][j] = mask_mult(2 * p + j, 0, wA[p][j])

            sA2 = [score_pair(0, 4 + 2 * p, 5 + 2 * p) for p in range(2)]
            wA2 = [exp_pair(0, 4 + 2 * p, 5 + 2 * p, *sA2[p]) for p in range(2)]
            for pr in range(4):
                v_transpose(pr)
            kts = list(range(8)) + [12, 13, 14, 15, 8, 9, 10, 11]
            wB = {}
            for p in range(4):                # slot B scores kt 0..7
                kt0, kt1 = kts[2 * p], kts[2 * p + 1]
                s0, s1 = score_pair(1, kt0, kt1)
                wB[p] = exp_pair(1, kt0, kt1, s0, s1)

            for p in range(2):                # slot A AV kt 0..3
                for j in range(2):
                    av_accum(avA_e, avA_o, 2 * p + j, wA[p][j],
                             2 * p + j == 0, False)

            kv_block(2)                       # K3 -> kvT 1536:2048
            for p in range(2):                # slot A AV kt 4..7 (covers the
                for j in range(2):            # kv_ps WAR window K3->K2)
                    av_accum(avA_e, avA_o, 4 + 2 * p + j, wA2[p][j],
                             False, 4 + 2 * p + j == 7)
            kv_block(3)                       # K2 -> kvT 1024:1536
            for p in (4, 5):                  # slot B scores kt 12..15
                kt0, kt1 = kts[2 * p], kts[2 * p + 1]
                s0, s1 = score_pair(1, kt0, kt1)
                wB[p] = exp_pair(1, kt0, kt1, s0, s1, defer_mult=True)
            for pr in (6, 7):
                v_transpose(pr)

            for p in (6, 7):                  # slot B scores kt 8..11
                kt0, kt1 = kts[2 * p], kts[2 * p + 1]
                s0, s1 = score_pair(1, kt0, kt1)
                wB[p] = exp_pair(1, kt0, kt1, s0, s1)
            for pr in (4, 5):
                v_transpose(pr)

            # slot A merge on DVE (ACT mid-exp); frees av pool for slot B
            oavA = ep.tile([H + 1, 512], DT, tag="oavA", name="oavA")
            ocA = ep.tile([H + 1, 512], F32, tag="ocA", name="ocA")
            for j in range(4):
                js = slice(j * 128, (j + 1) * 128)
                nc.vector.tensor_copy(ocA[:, js], avA_e[:, js])
                nc.vector.tensor_tensor(oavA[:, js], ocA[:, js],
                                        avA_o[:, js], add)
            # deferred slot-B diagonal mask mults (kt 12..15)
            for p in (4, 5):
                for j in range(2):
                    wB[p][j] = mask_mult(kts[2 * p + j], 1, wB[p][j])

            avB_e = avp.tile([H + 1, 512], F32, tag="avE", name="avE")
            avB_o = avp.tile([H + 1, 512], F32, tag="avO", name="avO")
            for p in range(4):                # slot B AV kt 0..7
                for j in range(2):
                    i = 2 * p + j
                    av_accum(avB_e, avB_o, kts[i], wB[p][j], i == 0, False)

            # slot A transpose + normalize + store
            for j in range(4):
                tr_ps = sp.tile([128, H + 1], DT, tag="score", name="otrA")
                nc.tensor.transpose(tr_ps[:], oavA[:, j * 128:(j + 1) * 128],
                                    id16_sb[:])
                r_sb = ep.tile([128, 1], F32, tag="recip", name="recip")
                nc.vector.reciprocal(r_sb[:], tr_ps[:, H:H + 1])
                o_col = j * H
                nc.vector.tensor_scalar_mul(
                    o_all[:, o_col:o_col + H], tr_ps[:, 0:H], r_sb[:])
            nc.sync.dma_start(out[:, 0:4 * H], o_all[:, 0:4 * H])

            for p in (4, 5, 6, 7):            # slot B AV kt 12..15, 8..11
                for j in range(2):
                    i = 2 * p + j
                    av_accum(avB_e, avB_o, kts[i], wB[p][j], False, i == 15)

            # slot B tail: merge on ACT (free after exps) + DVE
            oavB = ep.tile([H + 1, 512], DT, tag="oavB", name="oavB")
            ocB = ep.tile([H + 1, 512], F32, tag="ocB", name="ocB")
            for j in range(4):
                js = slice(j * 128, (j + 1) * 128)
                nc.scalar.activation(ocB[:, js], avB_e[:, js], Copy)
                nc.vector.tensor_tensor(oavB[:, js], ocB[:, js],
                                        avB_o[:, js], add)
                tr_ps = sp.tile([128, H + 1], DT, tag="score", name="otrB")
                nc.tensor.transpose(tr_ps[:], oavB[:, js], id16_sb[:])
                r_sb = ep.tile([128, 1], F32, tag="recip", name="recip")
                nc.vector.reciprocal(r_sb[:], tr_ps[:, H:H + 1])
                o_col = (4 + j) * H
                nc.vector.tensor_scalar_mul(
                    o_all[:, o_col:o_col + H], tr_ps[:, 0:H], r_sb[:])
            nc.scalar.dma_start(out[:, 4 * H:8 * H], o_all[:, 4 * H:8 * H])

            for pool in (avp, sp, pq, pk):
                pool.release()

    nc.compile()
    return nc


def _host_inputs(x, Wq, bq, Wk, bk, Wv, bv):
    """Build the 8 per-core input maps (all SBUF-layout, fp16/f32)."""
    f16 = np.float16
    Wkv = np.concatenate([Wk, Wv], axis=1)          # [D, 128]

    cst16_np = np.zeros((128, C16_N), dtype=f16)
    for d in range(8):
        cst16_np[:, C_WKV + d * 128:C_WKV + (d + 1) * 128] = \
            Wkv[d * 128:(d + 1) * 128, :]
        cst16_np[:, C_WQ + d * H:C_WQ + (d + 1) * H] = \
            Wq[d * 128:(d + 1) * 128, :]
    cst16_np[:, C_IDV:C_IDV + H] = np.concatenate(
        [np.eye(H), np.eye(H)], axis=0)
    cst16_np[0:H + 1, C_ID16:C_ID16 + H + 1] = np.eye(H + 1)
    xoff = {0: C16_N, 1: C16_N + 4096, 2: C16_N + 8192, 3: C16_N + 16384}

    in_maps = []
    for c in range(8):
        b = c // 2
        cA, cB = c % 2, 3 - c % 2
        perm = (cA, 1 - cA, 5 - cB, cB)        # chunk order along k
        xTp = np.concatenate(
            [x[b, p * CH:(p + 1) * CH].T for p in perm], axis=1)  # [D, S]
        xTp = xTp.astype(f16)
        blob_np = np.zeros((128, BLOB_N), dtype=f16)
        blob_np[:, 0:C16_N] = cst16_np
        for bi in range(4):
            kp = KPOS[bi]
            for d in range(8):
                o = xoff[bi] + d * 512
                blob_np[:, o:o + 512] = \
                    xTp[d * 128:(d + 1) * 128, kp:kp + 512]
        # k_global of permuted position p: perm[p//512]*512 + p%512
        pos = np.arange(S)
        kg = np.array(perm)[pos // CH] * CH + pos % CH
        thr_np = np.zeros((128, 2 * NKT), dtype=np.float32)
        p = np.arange(128)
        for slot, ck in enumerate((cA, cB)):
            for kt in range(NKT):
                thr_np[:, slot * NKT + kt] = kg[kt * 128 + p] - ck * CH
        thrb_np = np.zeros((128, 2 * NKT), dtype=np.float32)
        for slot in range(2):
            for kt in range(NKT):
                diag = (slot == 0 and kt < 4) or (slot == 1 and kt >= 12)
                if diag:
                    continue
                col = thr_np[:, slot * NKT + kt]
                if np.all(col <= 0):
                    continue          # fully visible -> bias 0
                thrb_np[:, slot * NKT + kt] = -1e5   # fully masked
        qio = np.arange(CH, dtype=np.float32)[None, :]
        for m in range(8):
            idx = m if m < 4 else NKT + 8 + m
            o = C16_N + 12288 + m * 512
            blob_np[:, o:o + 512] = \
                (qio >= thr_np[:, idx:idx + 1]).astype(f16)
        cst32_np = np.zeros((128, C32_N), dtype=np.float32)
        cst32_np[:, C_BKV] = np.concatenate([bk, bv])
        cst32_np[0:H, C_BQ] = bq
        cst32_np[:, C_THRB:C_THRB + 2 * NKT] = thrb_np
        in_maps.append({"blob": blob_np, "cst32": cst32_np})
    return in_maps


def _gather(results, dtype):
    y = np.zeros((B, S, H), dtype=dtype)
    for c in range(8):
        b = c // 2
        cA, cB = c % 2, 3 - c % 2
        o = results[c]["out"]
        for slot, ck in enumerate((cA, cB)):
            for j in range(4):
                col = (slot * 4 + j) * H
                y[b, ck * CH + j * 128:ck * CH + (j + 1) * 128] = \
                    o[:, col:col + H]
    return y


def get_nc():
    if "nc" not in _CACHE:
        _CACHE["nc"] = _build_nc()
    return _CACHE["nc"]


def kernel(x, Wq, bq, Wk, bk, Wv, bv, _trace=False, _trace_kwargs=None):
    from concourse.bass_utils import run_bass_kernel_spmd

    x = np.asarray(x, dtype=np.float32)
    Wq, bq = np.asarray(Wq, np.float32), np.asarray(bq, np.float32)
    Wk, bk = np.asarray(Wk, np.float32), np.asarray(bk, np.float32)
    Wv, bv = np.asarray(Wv, np.float32), np.asarray(bv, np.float32)

    nc = get_nc()
    in_maps = _host_inputs(x, Wq, bq, Wk, bk, Wv, bv)
    res = run_bass_kernel_spmd(
        nc, in_maps, core_ids=list(range(8)),
        trace=_trace, **(_trace_kwargs or {}))
    _CACHE["last_result"] = res
    return _gather(res.results, x.dtype)


# revision 20
# speedup vs baseline: 1.1003x; 1.0017x over previous
"""Trainium2 Bass kernel: single-head causal attention, SPMD over 8 NeuronCores.

Problem: x [4, 2048, 1024] f32; Wq/Wk/Wv [1024, 64]; bq/bk/bv [64].
  q,k,v = x@W + b ; out = softmax(causal(q k^T / 8)) @ v  -> [4, 2048, 64]

Sharding (uniform SPMD structure on every core):
  core c -> batch b = c//2 ; query chunks (cA, cB) = (c%2, 3-c%2), 512 rows
  each (pairing an early with a late chunk balances causal work).  Every core
  computes K/V for its batch's full 2048 rows.

Key layout trick: the k-axis is permuted PER CORE to chunk order
  [cA, 1-cA, 5-cB, cB], so the core's own query columns sit at STATIC
  positions 0:512 and 1536:2048 of the K/V input -- Q projection needs no
  separate input tensor.  Causality comes from host-precomputed per-core 0/1
  mask tiles (diagonal tiles) and exp-bias kills (fully-masked tiles), which
  absorb the permutation.

  Projections produce Q^T/K^T/V^T [64, rows]; scores are computed transposed
  ([k_part, q_free]) so the attention-weight matrix feeds the AV matmul as
  the moving operand; V is re-transposed through 16 small PE transposes; a
  65th "ones" row on the V tiles makes the AV matmul accumulate the softmax
  denominator for free.  Score matmuls (K=64) are row-packed in pairs into
  disjoint PE row-groups; the partition-64:127 K^T/Q^T duplicates they need
  are produced by partition-shifted bias-adds straight from PSUM (DVE/ACT
  read partitions 0:64, write 64:128) -- cheaper than SBUF->SBUF DMA
  (~0.65us sequencer issue + ~2us completion each).

Schedule: the hard resource is the ACT engine (24 exp tiles x ~0.58us,
  1 elem/cycle/lane regardless of dtype).  x streams K-MAJOR -- four
  1MB blocks of k-positions (K0=0:512, K1=512:1024, K3=1536:2048,
  K2=1024:1536), each holding all 8 d-tiles for those positions -- so the
  first K/V block (and the first scores-exp) is ready after ~1MB of
  stream + 16 matmuls instead of after the whole 2.5MB h0 half.  The exp
  chain then runs near-gapless while the remaining blocks stream/project.
  Inputs are FEW LARGE DMAs (one dma_start spans all 16 SDMA engines at
  ~340GB/s; issue costs ~0.65us sequencer time each, so count is key) in
  priority order on the sync queue.  Slot-A attention interleaves with the
  later projections on the PE (PSUM: proj kv 1 + q 2 + score 3 + av 2 = 8
  banks; the 2-bank av pool is reused A->B).  Slot-A merge runs on DVE
  (ACT is mid-exp); slot-B merge on ACT (free after the last exp).

dtypes: fp16 SBUF operands, fp32 PSUM accumulation, fp32 normalize.
"""

import os
import sys

import numpy as np

if "/opt/trn_rl_repo" not in sys.path:
    sys.path.insert(0, "/opt/trn_rl_repo")

B, S, D, H = 4, 2048, 1024, 64
CH = 512          # query chunk width
QR = 2 * CH       # query rows per core
NKT = S // 128    # 16 k-tiles of 128
SCALE = 1.0 / np.sqrt(H)

# fp16 blob column layout -- weights/identities at the head
C_WKV = 0          # [128, 1024] 8 d blocks of [128, 128]
C_WQ = 1024        # [128, 512]  8 d blocks of [128, 64]
C_IDV = 1536       # [128, 64]   eye(64) stacked twice
C_ID16 = 1600      # [65, 65]    eye(65)
C16_N = 1696       # padded total
# blob chunk map (name, col0, col1): DMA/stream order
BLOB_CH = [("hd", 0, C16_N + 2048),               # cst16 + K0 d0..3
           ("k0b", C16_N + 2048, C16_N + 4096),   # K0 d4..7
           ("k1", C16_N + 4096, C16_N + 8192),
           ("k3", C16_N + 8192, C16_N + 12288),
           ("msk", C16_N + 12288, C16_N + 16384),
           ("k2", C16_N + 16384, C16_N + 20480)]
BLOB_N = C16_N + 20480

# cst32 column layout (f32)
C_BKV = 0          # [128, 1]
C_BQ = 1           # [64, 1]
C_THRB = 2         # [128, 32] exp bias: 0 or -1e5 (fully-masked kill)
C32_N = 34

# xk k-major block layout: block bi covers k-positions KPOS[bi]:+512 with
# 8 d-tiles of [128, 512] each; stream order K0, K1, K3, K2.
KPOS = (0, 512, 1536, 1024)
# kvT 512-col block index (nb) per stream block
KNB = (0, 1, 3, 2)

_CACHE = {}


def _build_nc():
    import concourse.bacc as bacc
    import concourse.mybir as mybir
    import concourse.tile as tile

    DT = mybir.dt.float16
    F32 = mybir.dt.float32
    Exp = mybir.ActivationFunctionType.Exp
    Copy = mybir.ActivationFunctionType.Copy
    Ident = mybir.ActivationFunctionType.Identity
    mult = mybir.AluOpType.mult
    add = mybir.AluOpType.add

    nc = bacc.Bacc("TRN2", target_bir_lowering=False, debug=False, num_devices=8)

    blob = nc.dram_tensor("blob", [128, BLOB_N], DT, kind="ExternalInput")
    cst32 = nc.dram_tensor("cst32", [128, C32_N], F32, kind="ExternalInput")
    out = nc.dram_tensor("out", [128, 8 * H], DT, kind="ExternalOutput")

    with tile.TileContext(nc) as tc:
        with (
            tc.tile_pool(name="const", bufs=1) as cp,
            tc.tile_pool(name="work", bufs=16) as wp,
            tc.tile_pool(name="epi", bufs=4) as ep,
        ):
            # ---- head: 6 large DMAs in priority order on the sync
            # queue; weights+K0a share ONE DMA (each extra dma_start on the
            # critical path costs ~1us issue+drain-start lag)
            tiles = {}
            for nm, c0, c1 in BLOB_CH:
                tiles[nm] = cp.tile([128, c1 - c0], DT, tag=nm, name=nm)
                nc.sync.dma_start(tiles[nm][:], blob[:, c0:c1])
            hd_sb, xb0b = tiles["hd"], tiles["k0b"]
            xb1, xb3, xb2 = tiles["k1"], tiles["k3"], tiles["k2"]
            msk_sb = tiles["msk"]
            cst32_sb = cp.tile([128, C32_N], F32, tag="cst32", name="cst32")
            nc.gpsimd.dma_start(cst32_sb[:], cst32[:])

            wkv_sb = hd_sb[:, C_WKV:C_WKV + 1024]
            wq_sb = hd_sb[:, C_WQ:C_WQ + 512]
            idv_sb = hd_sb[:, C_IDV:C_IDV + H]
            id16_sb = hd_sb[0:H + 1, C_ID16:C_ID16 + H + 1]
            bkv_sb = cst32_sb[:, C_BKV:C_BKV + 1]
            bq_sb = cst32_sb[0:H, C_BQ:C_BQ + 1]
            thrb_sb = cst32_sb[:, C_THRB:C_THRB + 2 * NKT]

            def xs(bi, d):    # stream block bi, d-tile d -> [128, 512]
                if bi == 0:
                    if d < 4:
                        return hd_sb[:, C16_N + d * 512:C16_N + (d + 1) * 512]
                    return xb0b[:, (d - 4) * 512:(d - 3) * 512]
                t = (xb1, xb3, xb2)[bi - 1]
                return t[:, d * 512:(d + 1) * 512]

            kvT_sb = cp.tile([128, S], DT, tag="kvT", name="kvT")  # 0:64 K^T, 64:128 V^T
            qT_sb = cp.tile([H, QR], DT, tag="qT", name="qT")      # A cols 0:512, B 512:1024
            # V tiles at stride 80 cols (160B): dma_start_transpose needs
            # 32-byte-aligned SBUF destinations (65*2=130B strides are not)
            v_sb = cp.tile([128, NKT * 80], DT, tag="v", name="v")
            # duplicates at partitions 64:127 for row-packed score pairs
            ktd_sb = cp.tile([128, S], DT, tag="ktd", name="ktd")
            qTd_sb = cp.tile([128, QR], DT, tag="qTd", name="qTd")
            o_all = cp.tile([128, 8 * H], DT, tag="oall", name="oall")

            nc.vector.memset(v_sb[:], 1.0)

            pk = tc.alloc_tile_pool(name="proj_ps", bufs=1, space="PSUM")
            pq = tc.alloc_tile_pool(name="q_ps", bufs=1, space="PSUM")
            sp = tc.alloc_tile_pool(name="score_ps", bufs=3, space="PSUM")
            avp = tc.alloc_tile_pool(name="av_ps", bufs=1, space="PSUM")

            def v_dma(kt):
                # V tile via HWDGE xbar DMA-transpose straight out of kvT --
                # replaces a PE transpose + DVE copy; sync queue only (an
                # issue slot on the scalar queue would stall the exp spine)
                nc.sync.dma_start_transpose(
                    v_sb[:, kt * 80:kt * 80 + H],
                    kvT_sb[64:128, kt * 128:(kt + 1) * 128])

            def kv_epi(cs, kv_ps):
                # ktd dup first: it unblocks the row-packed scores
                nc.vector.tensor_scalar(ktd_sb[H:128, cs], kv_ps[0:H, :],
                                        bkv_sb[0:H, :], None, add)
                nc.vector.tensor_scalar(kvT_sb[:, cs], kv_ps[:],
                                        bkv_sb[:], None, add)

            def kv_block(bi):
                """8 accumulating matmuls for one 512-position k block + DVE
                epilogue + V-tile transposes."""
                kv_ps = pk.tile([128, 512], F32, tag="kvps", name="kvps")
                for d in range(8):
                    nc.tensor.matmul(
                        kv_ps[:], wkv_sb[:, d * 128:(d + 1) * 128],
                        xs(bi, d), start=(d == 0), stop=(d == 7))
                nb = KNB[bi]
                kv_epi(slice(nb * 512, (nb + 1) * 512), kv_ps)
                for kt in range(nb * 4, nb * 4 + 4):
                    v_dma(kt)

            def kv_half(bi, half):
                """256-position half block: halves the latency from x-block
                arrival to the first usable kvT columns."""
                kv_ps = pk.tile([128, 256], F32, tag="kvh", name="kvh")
                for d in range(8):
                    nc.tensor.matmul(
                        kv_ps[:], wkv_sb[:, d * 128:(d + 1) * 128],
                        xs(bi, d)[:, half * 256:(half + 1) * 256],
                        start=(d == 0), stop=(d == 7))
                nb = KNB[bi]
                kv_epi(slice(nb * 512 + half * 256,
                             nb * 512 + (half + 1) * 256), kv_ps)
                for kt in range(nb * 4 + half * 2, nb * 4 + half * 2 + 2):
                    v_dma(kt)

            def q_block(bi, slot):
                q_ps = pq.tile([H, 512], F32, tag="qps", name="qps")
                for d in range(8):
                    nc.tensor.matmul(
                        q_ps[:], wq_sb[:, d * H:(d + 1) * H],
                        xs(bi, d), start=(d == 0), stop=(d == 7))
                cs = slice(slot * 512, (slot + 1) * 512)
                if slot == 0:     # ACT is idle pre-exp; keep DVE for kv epis
                    nc.scalar.activation(qT_sb[:, cs], q_ps[:], Ident,
                                         bias=bq_sb[:])
                    nc.scalar.activation(qTd_sb[H:128, cs], q_ps[:], Ident,
                                         bias=bq_sb[:])
                else:             # ACT is mid-exp by now
                    nc.vector.tensor_scalar(qT_sb[:, cs], q_ps[:],
                                            bq_sb[:], None, add)
                    nc.vector.tensor_scalar(qTd_sb[H:128, cs], q_ps[:],
                                            bq_sb[:], None, add)

            def score_pair(slot, kt0, kt1):
                s0 = sp.tile([128, 512], F32, tag="score", name="score0")
                nc.tensor.matmul(
                    s0[:], kvT_sb[0:H, kt0 * 128:(kt0 + 1) * 128],
                    qT_sb[:, slot * 512:(slot + 1) * 512],
                    start=True, stop=True, tile_position=(0, 0))
                s1 = sp.tile([128, 512], F32, tag="score", name="score1")
                nc.tensor.matmul(
                    s1[:], ktd_sb[H:128, kt1 * 128:(kt1 + 1) * 128],
                    qTd_sb[H:128, slot * 512:(slot + 1) * 512],
                    start=True, stop=True, tile_position=(64, 0))
                return s0, s1

            def mask_mult(kt, slot, w_sb):
                m = kt if slot == 0 else kt - 8
                wm_sb = wp.tile([128, 512], DT, tag="wm", name="wm")
                nc.vector.tensor_tensor(
                    wm_sb[:], w_sb[:], msk_sb[:, m * 512:(m + 1) * 512], mult)
                return wm_sb

            def exp_pair(slot, kt0, kt1, s0, s1, defer_mult=False):
                """exps on ACT; diag tiles get a DVE mask multiply, which can
                be deferred so exp-gated mults don't clog the DVE queue
                ahead of spine-critical kv epilogues."""
                res = []
                for kt, s_ps in zip((kt0, kt1), (s0, s1)):
                    idx = slot * NKT + kt
                    w_sb = wp.tile([128, 512], DT, tag="wexp", name="wexp")
                    nc.scalar.activation(w_sb[:], s_ps[:], Exp,
                                         bias=thrb_sb[:, idx:idx + 1],
                                         scale=float(SCALE))
                    diag = (slot == 0 and kt < 4) or (slot == 1 and kt >= 12)
                    if diag and not defer_mult:
                        res.append(mask_mult(kt, slot, w_sb))
                    else:
                        res.append(w_sb)
                return res

            def av_accum(av_e, av_o, kt, w_av, first, last):
                vs = slice(kt * 80, kt * 80 + H + 1)
                nc.tensor.matmul(
                    av_e[:], v_sb[0:H, vs], w_av[0:H, :],
                    start=first, stop=last, tile_position=(0, 0))
                nc.tensor.matmul(
                    av_o[:], v_sb[H:128, vs], w_av[H:128, :],
                    start=first, stop=last, tile_position=(64, 0))

            # ================= emission (PE-queue order) =================
            q_block(0, 0)                     # qA (+ ACT idents)
            avA_e = avp.tile([H + 1, 512], F32, tag="avE", name="avE")
            avA_o = avp.tile([H + 1, 512], F32, tag="avO", name="avO")
            sA, wA, sA2, wA2 = [], [], [], []
            for p in range(2):                # K0 halves -> scores kt0..3
                kv_half(0, p)
                s = score_pair(0, 2 * p, 2 * p + 1)
                sA.append(s)
                wA.append(exp_pair(0, 2 * p, 2 * p + 1, *s, defer_mult=True))
            for p in range(2):                # K1 halves -> scores kt4..7
                kv_half(1, p)
                s = score_pair(0, 4 + 2 * p, 5 + 2 * p)
                sA2.append(s)
                wA2.append(exp_pair(0, 4 + 2 * p, 5 + 2 * p, *s))

            q_block(2, 1)                     # qB: its DVE epilogue (qTd-B)
            # gates the slot-B half of the exp spine
            for p in range(2):
                for j in range(2):
                    wA[p][j] = mask_mult(2 * p + j, 0, wA[p][j])
            kts = list(range(8)) + [12, 13, 14, 15, 8, 9, 10, 11]
            wB = {}
            for p in range(4):                # slot B scores kt 0..7
                kt0, kt1 = kts[2 * p], kts[2 * p + 1]
                s0, s1 = score_pair(1, kt0, kt1)
                wB[p] = exp_pair(1, kt0, kt1, s0, s1)

            for p in range(2):                # slot A AV kt 0..3
                for j in range(2):
                    av_accum(avA_e, avA_o, 2 * p + j, wA[p][j],
                             2 * p + j == 0, False)

            kv_block(2)                       # K3 -> kvT 1536:2048
            for p in range(2):                # slot A AV kt 4..7 (covers the
                for j in range(2):            # kv_ps WAR window K3->K2)
                    av_accum(avA_e, avA_o, 4 + 2 * p + j, wA2[p][j],
                             False, 4 + 2 * p + j == 7)
            kv_block(3)                       # K2 -> kvT 1024:1536
            for p in (4, 5):                  # slot B scores kt 12..15
                kt0, kt1 = kts[2 * p], kts[2 * p + 1]
                s0, s1 = score_pair(1, kt0, kt1)
                wB[p] = exp_pair(1, kt0, kt1, s0, s1, defer_mult=True)
            for p in (6, 7):                  # slot B scores kt 8..11
                kt0, kt1 = kts[2 * p], kts[2 * p + 1]
                s0, s1 = score_pair(1, kt0, kt1)
                wB[p] = exp_pair(1, kt0, kt1, s0, s1)
            # slot A merge on DVE (ACT mid-exp); frees av pool for slot B
            oavA = ep.tile([H + 1, 512], DT, tag="oavA", name="oavA")
            ocA = ep.tile([H + 1, 512], F32, tag="ocA", name="ocA")
            for j in range(4):
                js = slice(j * 128, (j + 1) * 128)
                nc.vector.tensor_copy(ocA[:, js], avA_e[:, js])
                nc.vector.tensor_tensor(oavA[:, js], ocA[:, js],
                                        avA_o[:, js], add)
            # deferred slot-B diagonal mask mults (kt 12..15)
            for p in (4, 5):
                for j in range(2):
                    wB[p][j] = mask_mult(kts[2 * p + j], 1, wB[p][j])

            avB_e = avp.tile([H + 1, 512], F32, tag="avE", name="avE")
            avB_o = avp.tile([H + 1, 512], F32, tag="avO", name="avO")
            for p in range(4):                # slot B AV kt 0..7
                for j in range(2):
                    i = 2 * p + j
                    av_accum(avB_e, avB_o, kts[i], wB[p][j], i == 0, False)

            # slot A transpose + normalize + store
            for j in range(4):
                tr_ps = sp.tile([128, H + 1], DT, tag="score", name="otrA")
                nc.tensor.transpose(tr_ps[:], oavA[:, j * 128:(j + 1) * 128],
                                    id16_sb[:])
                r_sb = ep.tile([128, 1], F32, tag="recip", name="recip")
                nc.vector.reciprocal(r_sb[:], tr_ps[:, H:H + 1])
                o_col = j * H
                nc.vector.tensor_scalar_mul(
                    o_all[:, o_col:o_col + H], tr_ps[:, 0:H], r_sb[:])
            nc.sync.dma_start(out[:, 0:4 * H], o_all[:, 0:4 * H])

            for p in (4, 5, 6, 7):            # slot B AV kt 12..15, 8..11
                for j in range(2):
                    i = 2 * p + j
                    av_accum(avB_e, avB_o, kts[i], wB[p][j], False, i == 15)

            # slot B tail: merge on ACT (free after exps) + DVE
            oavB = ep.tile([H + 1, 512], DT, tag="oavB", name="oavB")
            ocB = ep.tile([H + 1, 512], F32, tag="ocB", name="ocB")
            for j in range(4):
                js = slice(j * 128, (j + 1) * 128)
                nc.scalar.activation(ocB[:, js], avB_e[:, js], Copy)
                nc.vector.tensor_tensor(oavB[:, js], ocB[:, js],
                                        avB_o[:, js], add)
                tr_ps = sp.tile([128, H + 1], DT, tag="score", name="otrB")
                nc.tensor.transpose(tr_ps[:], oavB[:, js], id16_sb[:])
                r_sb = ep.tile([128, 1], F32, tag="recip", name="recip")
                nc.vector.reciprocal(r_sb[:], tr_ps[:, H:H + 1])
                o_col = (4 + j) * H
                nc.vector.tensor_scalar_mul(
                    o_all[:, o_col:o_col + H], tr_ps[:, 0:H], r_sb[:])
            nc.scalar.dma_start(out[:, 4 * H:8 * H], o_all[:, 4 * H:8 * H])

            for pool in (avp, sp, pq, pk):
                pool.release()

    nc.compile()
    return nc


def _host_inputs(x, Wq, bq, Wk, bk, Wv, bv):
    """Build the 8 per-core input maps (all SBUF-layout, fp16/f32)."""
    f16 = np.float16
    Wkv = np.concatenate([Wk, Wv], axis=1)          # [D, 128]

    cst16_np = np.zeros((128, C16_N), dtype=f16)
    for d in range(8):
        cst16_np[:, C_WKV + d * 128:C_WKV + (d + 1) * 128] = \
            Wkv[d * 128:(d + 1) * 128, :]
        cst16_np[:, C_WQ + d * H:C_WQ + (d + 1) * H] = \
            Wq[d * 128:(d + 1) * 128, :]
    cst16_np[:, C_IDV:C_IDV + H] = np.concatenate(
        [np.eye(H), np.eye(H)], axis=0)
    cst16_np[0:H + 1, C_ID16:C_ID16 + H + 1] = np.eye(H + 1)
    xoff = {0: C16_N, 1: C16_N + 4096, 2: C16_N + 8192, 3: C16_N + 16384}

    in_maps = []
    for c in range(8):
        b = c // 2
        cA, cB = c % 2, 3 - c % 2
        perm = (cA, 1 - cA, 5 - cB, cB)        # chunk order along k
        xTp = np.concatenate(
            [x[b, p * CH:(p + 1) * CH].T for p in perm], axis=1)  # [D, S]
        xTp = xTp.astype(f16)
        blob_np = np.zeros((128, BLOB_N), dtype=f16)
        blob_np[:, 0:C16_N] = cst16_np
        for bi in range(4):
            kp = KPOS[bi]
            for d in range(8):
                o = xoff[bi] + d * 512
                blob_np[:, o:o + 512] = \
                    xTp[d * 128:(d + 1) * 128, kp:kp + 512]
        # k_global of permuted position p: perm[p//512]*512 + p%512
        pos = np.arange(S)
        kg = np.array(perm)[pos // CH] * CH + pos % CH
        thr_np = np.zeros((128, 2 * NKT), dtype=np.float32)
        p = np.arange(128)
        for slot, ck in enumerate((cA, cB)):
            for kt in range(NKT):
                thr_np[:, slot * NKT + kt] = kg[kt * 128 + p] - ck * CH
        thrb_np = np.zeros((128, 2 * NKT), dtype=np.float32)
        for slot in range(2):
            for kt in range(NKT):
                diag = (slot == 0 and kt < 4) or (slot == 1 and kt >= 12)
                if diag:
                    continue
                col = thr_np[:, slot * NKT + kt]
                if np.all(col <= 0):
                    continue          # fully visible -> bias 0
                thrb_np[:, slot * NKT + kt] = -1e5   # fully masked
        qio = np.arange(CH, dtype=np.float32)[None, :]
        for m in range(8):
            idx = m if m < 4 else NKT + 8 + m
            o = C16_N + 12288 + m * 512
            blob_np[:, o:o + 512] = \
                (qio >= thr_np[:, idx:idx + 1]).astype(f16)
        cst32_np = np.zeros((128, C32_N), dtype=np.float32)
        cst32_np[:, C_BKV] = np.concatenate([bk, bv])
        cst32_np[0:H, C_BQ] = bq
        cst32_np[:, C_THRB:C_THRB + 2 * NKT] = thrb_np
        in_maps.append({"blob": blob_np, "cst32": cst32_np})
    return in_maps


def _gather(results, dtype):
    y = np.zeros((B, S, H), dtype=dtype)
    for c in range(8):
        b = c // 2
        cA, cB = c % 2, 3 - c % 2
        o = results[c]["out"]
        for slot, ck in enumerate((cA, cB)):
            for j in range(4):
                col = (slot * 4 + j) * H
                y[b, ck * CH + j * 128:ck * CH + (j + 1) * 128] = \
                    o[:, col:col + H]
    return y


def get_nc():
    if "nc" not in _CACHE:
        _CACHE["nc"] = _build_nc()
    return _CACHE["nc"]


def kernel(x, Wq, bq, Wk, bk, Wv, bv, _trace=False, _trace_kwargs=None):
    from concourse.bass_utils import run_bass_kernel_spmd

    x = np.asarray(x, dtype=np.float32)
    Wq, bq = np.asarray(Wq, np.float32), np.asarray(bq, np.float32)
    Wk, bk = np.asarray(Wk, np.float32), np.asarray(bk, np.float32)
    Wv, bv = np.asarray(Wv, np.float32), np.asarray(bv, np.float32)

    nc = get_nc()
    in_maps = _host_inputs(x, Wq, bq, Wk, bk, Wv, bv)
    res = run_bass_kernel_spmd(
        nc, in_maps, core_ids=list(range(8)),
        trace=_trace, **(_trace_kwargs or {}))
    _CACHE["last_result"] = res
    return _gather(res.results, x.dtype)


# revision 24
# speedup vs baseline: 1.2138x; 1.1032x over previous
"""Trainium2 Bass kernel: single-head causal attention, SPMD over 8 NeuronCores.

Problem: x [4, 2048, 1024] f32; Wq/Wk/Wv [1024, 64]; bq/bk/bv [64].
  q,k,v = x@W + b ; out = softmax(causal(q k^T / 8)) @ v  -> [4, 2048, 64]

Sharding (uniform SPMD structure on every core):
  core c -> batch b = c//2 ; query chunks (cA, cB) = (c%2, 3-c%2), 512 rows
  each (pairing an early with a late chunk balances causal work).  Every core
  computes K/V for its batch's full 2048 rows.

Key layout trick: the k-axis is permuted PER CORE to chunk order
  [cA, 1-cA, 5-cB, cB], so the core's own query columns sit at STATIC
  positions 0:512 and 1536:2048 of the K/V input -- Q projection needs no
  separate input tensor.  Causality comes from host-precomputed per-core 0/1
  mask tiles (diagonal tiles) and exp-bias kills (fully-masked tiles), which
  absorb the permutation.

  Projections produce Q^T/K^T/V^T [64, rows]; scores are computed transposed
  ([k_part, q_free]) so the attention-weight matrix feeds the AV matmul as
  the moving operand; V is re-transposed through 16 small PE transposes; a
  65th "ones" row on the V tiles makes the AV matmul accumulate the softmax
  denominator for free.  Score matmuls (K=64) are row-packed in pairs into
  disjoint PE row-groups; the partition-64:127 K^T/Q^T duplicates they need
  are produced by partition-shifted bias-adds straight from PSUM (DVE/ACT
  read partitions 0:64, write 64:128) -- cheaper than SBUF->SBUF DMA
  (~0.65us sequencer issue + ~2us completion each).

Schedule: the hard resource is the ACT engine (24 exp tiles x ~0.58us,
  1 elem/cycle/lane regardless of dtype).  x streams K-MAJOR -- four
  1MB blocks of k-positions (K0=0:512, K1=512:1024, K3=1536:2048,
  K2=1024:1536), each holding all 8 d-tiles for those positions -- so the
  first K/V block (and the first scores-exp) is ready after ~1MB of
  stream + 16 matmuls instead of after the whole 2.5MB h0 half.  The exp
  chain then runs near-gapless while the remaining blocks stream/project.
  Inputs are FEW LARGE DMAs (one dma_start spans all 16 SDMA engines at
  ~340GB/s; issue costs ~0.65us sequencer time each, so count is key) in
  priority order on the sync queue.  Slot-A attention interleaves with the
  later projections on the PE (PSUM: proj kv 1 + q 2 + score 3 + av 2 = 8
  banks; the 2-bank av pool is reused A->B).  Slot-A merge runs on DVE
  (ACT is mid-exp); slot-B merge on ACT (free after the last exp).

dtypes: fp16 SBUF operands, fp32 PSUM accumulation, fp32 normalize.
"""

import os
import sys

import numpy as np

if "/opt/trn_rl_repo" not in sys.path:
    sys.path.insert(0, "/opt/trn_rl_repo")

B, S, D, H = 4, 2048, 1024, 64
CH = 512          # query chunk width
QR = 2 * CH       # query rows per core
NKT = S // 128    # 16 k-tiles of 128
SCALE = 1.0 / np.sqrt(H)

# fp16 blob column layout -- weights/identities at the head
C_WKV = 0          # [128, 1024] 8 d blocks of [128, 128]
C_WQ = 1024        # [128, 512]  8 d blocks of [128, 64]
C_IDV = 1536       # [128, 64]   eye(64) stacked twice
C_ID16 = 1600      # [65, 65]    eye(65)
C16_N = 1696       # padded total
# blob chunk map (name, col0, col1): DMA/stream order
BLOB_CH = [("hd", 0, C16_N + 2048),               # cst16 + K0 d0..3
           ("k0b", C16_N + 2048, C16_N + 4096),   # K0 d4..7
           ("k1", C16_N + 4096, C16_N + 8192),
           ("k3", C16_N + 8192, C16_N + 12288),
           ("msk", C16_N + 12288, C16_N + 16384),
           ("k2", C16_N + 16384, C16_N + 20480)]
BLOB_N = C16_N + 20480

# cst32 column layout (f32)
C_BKV = 0          # [128, 1]
C_BQ = 1           # [64, 1]
C_THRB = 2         # [128, 32] exp bias: 0 or -1e5 (fully-masked kill)
C32_N = 34

# xk k-major block layout: block bi covers k-positions KPOS[bi]:+512 with
# 8 d-tiles of [128, 512] each; stream order K0, K1, K3, K2.
KPOS = (0, 512, 1536, 1024)
# kvT 512-col block index (nb) per stream block
KNB = (0, 1, 3, 2)

_CACHE = {}


def _build_nc():
    import concourse.bacc as bacc
    import concourse.mybir as mybir
    import concourse.tile as tile

    DT = mybir.dt.float16
    F32 = mybir.dt.float32
    Exp = mybir.ActivationFunctionType.Exp
    Copy = mybir.ActivationFunctionType.Copy
    Ident = mybir.ActivationFunctionType.Identity
    mult = mybir.AluOpType.mult
    add = mybir.AluOpType.add

    nc = bacc.Bacc("TRN2", target_bir_lowering=False, debug=False, num_devices=8)

    blob = nc.dram_tensor("blob", [128, BLOB_N], DT, kind="ExternalInput")
    cst32 = nc.dram_tensor("cst32", [128, C32_N], F32, kind="ExternalInput")
    out = nc.dram_tensor("out", [128, 8 * H], DT, kind="ExternalOutput")

    with tile.TileContext(nc) as tc:
        with (
            tc.tile_pool(name="const", bufs=1) as cp,
            tc.tile_pool(name="work", bufs=16) as wp,
            tc.tile_pool(name="epi", bufs=4) as ep,
        ):
            # ---- head: 6 large DMAs in priority order on the sync
            # queue; weights+K0a share ONE DMA (each extra dma_start on the
            # critical path costs ~1us issue+drain-start lag)
            tiles = {}
            for nm, c0, c1 in BLOB_CH:
                tiles[nm] = cp.tile([128, c1 - c0], DT, tag=nm, name=nm)
                nc.sync.dma_start(tiles[nm][:], blob[:, c0:c1])
            hd_sb, xb0b = tiles["hd"], tiles["k0b"]
            xb1, xb3, xb2 = tiles["k1"], tiles["k3"], tiles["k2"]
            msk_sb = tiles["msk"]
            cst32_sb = cp.tile([128, C32_N], F32, tag="cst32", name="cst32")
            nc.gpsimd.dma_start(cst32_sb[:], cst32[:])

            wkv_sb = hd_sb[:, C_WKV:C_WKV + 1024]
            wq_sb = hd_sb[:, C_WQ:C_WQ + 512]
            idv_sb = hd_sb[:, C_IDV:C_IDV + H]
            id16_sb = hd_sb[0:H + 1, C_ID16:C_ID16 + H + 1]
            bkv_sb = cst32_sb[:, C_BKV:C_BKV + 1]
            bq_sb = cst32_sb[0:H, C_BQ:C_BQ + 1]
            thrb_sb = cst32_sb[:, C_THRB:C_THRB + 2 * NKT]

            def xs(bi, d):    # stream block bi, d-tile d -> [128, 512]
                if bi == 0:
                    if d < 4:
                        return hd_sb[:, C16_N + d * 512:C16_N + (d + 1) * 512]
                    return xb0b[:, (d - 4) * 512:(d - 3) * 512]
                t = (xb1, xb3, xb2)[bi - 1]
                return t[:, d * 512:(d + 1) * 512]

            kvT_sb = cp.tile([128, S], DT, tag="kvT", name="kvT")  # 0:64 K^T, 64:128 V^T
            qT_sb = cp.tile([H, QR], DT, tag="qT", name="qT")      # A cols 0:512, B 512:1024
            v_sb = cp.tile([128, NKT * (H + 1)], DT, tag="v", name="v")
            vtd_sb = cp.tile([64, S], DT, tag="vtd", name="vtd")
            o_all = cp.tile([128, 8 * H], DT, tag="oall", name="oall")

            nc.vector.memset(v_sb[:], 1.0)

            pk = tc.alloc_tile_pool(name="proj_ps", bufs=1, space="PSUM")
            pq = tc.alloc_tile_pool(name="q_ps", bufs=1, space="PSUM")
            sp = tc.alloc_tile_pool(name="score_ps", bufs=4, space="PSUM")
            avp = tc.alloc_tile_pool(name="av_ps", bufs=1, space="PSUM")

            def kv_block(bi):
                """8 accumulating matmuls for one 512-position k block, then
                DVE epilogue (ktd dup first: it unblocks row-packed scores)."""
                kv_ps = pk.tile([128, 512], F32, tag="kvps", name="kvps")
                for d in range(8):
                    nc.tensor.matmul(
                        kv_ps[:], wkv_sb[:, d * 128:(d + 1) * 128],
                        xs(bi, d), start=(d == 0), stop=(d == 7))
                nb = KNB[bi]
                cs = slice(nb * 512, (nb + 1) * 512)
                nc.vector.tensor_scalar(kvT_sb[:, cs], kv_ps[:],
                                        bkv_sb[:], None, add)
                nc.vector.tensor_scalar(vtd_sb[:, cs], kv_ps[H:128, :],
                                        bkv_sb[H:128, :], None, add)

            def q_block(bi, slot):
                q_ps = pq.tile([H, 512], F32, tag="qps", name="qps")
                for d in range(8):
                    nc.tensor.matmul(
                        q_ps[:], wq_sb[:, d * H:(d + 1) * H],
                        xs(bi, d), start=(d == 0), stop=(d == 7))
                cs = slice(slot * 512, (slot + 1) * 512)
                if slot == 0:     # ACT is idle pre-exp; keep DVE for kv epis
                    nc.scalar.activation(qT_sb[:, cs], q_ps[:], Ident,
                                         bias=bq_sb[:])
                else:             # ACT is mid-exp by now
                    nc.vector.tensor_scalar(qT_sb[:, cs], q_ps[:],
                                            bq_sb[:], None, add)

            def v_transpose(pr):
                k0, k1 = 2 * pr, 2 * pr + 1
                t0 = sp.tile([128, H], DT, tag="score", name="vtr0")
                nc.tensor.transpose(
                    t0[:], vtd_sb[:, k0 * 128:(k0 + 1) * 128],
                    idv_sb[0:H, :], tile_position=(0, 0))
                t1 = sp.tile([128, H], DT, tag="score", name="vtr1")
                nc.tensor.transpose(
                    t1[:], kvT_sb[64:128, k1 * 128:(k1 + 1) * 128],
                    idv_sb[64:64 + H, :], tile_position=(64, 0))
                nc.vector.tensor_copy(
                    v_sb[:, k0 * (H + 1):k0 * (H + 1) + H], t0[:])
                nc.vector.tensor_copy(
                    v_sb[:, k1 * (H + 1):k1 * (H + 1) + H], t1[:])

            def score_pair(slot, kt0, kt1):
                s0 = sp.tile([128, 512], F32, tag="score", name="score0")
                nc.tensor.matmul(
                    s0[:], kvT_sb[0:H, kt0 * 128:(kt0 + 1) * 128],
                    qT_sb[:, slot * 512:(slot + 1) * 512],
                    start=True, stop=True)
                s1 = sp.tile([128, 512], F32, tag="score", name="score1")
                nc.tensor.matmul(
                    s1[:], kvT_sb[0:H, kt1 * 128:(kt1 + 1) * 128],
                    qT_sb[:, slot * 512:(slot + 1) * 512],
                    start=True, stop=True)
                return s0, s1

            def mask_mult(kt, slot, w_sb):
                m = kt if slot == 0 else kt - 8
                wm_sb = wp.tile([128, 512], DT, tag="wm", name="wm")
                nc.vector.tensor_tensor(
                    wm_sb[:], w_sb[:], msk_sb[:, m * 512:(m + 1) * 512], mult)
                return wm_sb

            def exp_pair(slot, kt0, kt1, s0, s1, defer_mult=False):
                """exps on ACT; diag tiles get a DVE mask multiply, which can
                be deferred so exp-gated mults don't clog the DVE queue
                ahead of spine-critical kv epilogues."""
                res = []
                for kt, s_ps in zip((kt0, kt1), (s0, s1)):
                    idx = slot * NKT + kt
                    w_sb = wp.tile([128, 512], DT, tag="wexp", name="wexp")
                    nc.scalar.activation(w_sb[:], s_ps[:], Exp,
                                         bias=thrb_sb[:, idx:idx + 1],
                                         scale=float(SCALE))
                    diag = (slot == 0 and kt < 4) or (slot == 1 and kt >= 12)
                    if diag and not defer_mult:
                        res.append(mask_mult(kt, slot, w_sb))
                    else:
                        res.append(w_sb)
                return res

            def av_accum(av_e, av_o, kt, w_av, first, last):
                vs = slice(kt * (H + 1), (kt + 1) * (H + 1))
                nc.tensor.matmul(
                    av_e[:], v_sb[0:H, vs], w_av[0:H, :],
                    start=first, stop=last, tile_position=(0, 0))
                nc.tensor.matmul(
                    av_o[:], v_sb[H:128, vs], w_av[H:128, :],
                    start=first, stop=last, tile_position=(64, 0))

            # ================= emission (PE-queue order) =================
            kv_block(0)                       # K0 -> kvT 0:512
            q_block(0, 0)                     # qA (+ ACT idents)
            kv_block(1)                       # K1 -> kvT 512:1024
            # (kv1's DVE epilogue queues BEFORE slot-A mults/v-copies so
            # the kt4..7 scores are never DVE-starved)

            avA_e = avp.tile([H + 1, 512], F32, tag="avE", name="avE")
            avA_o = avp.tile([H + 1, 512], F32, tag="avO", name="avO")
            sA = [score_pair(0, 2 * p, 2 * p + 1) for p in range(2)]
            wA = [exp_pair(0, 2 * p, 2 * p + 1, *sA[p], defer_mult=True)
                  for p in range(2)]

            q_block(2, 1)                     # qB early: its DVE epilogue
            # (qTd-B) gates the slot-B half of the exp spine
            for p in range(2):
                for j in range(2):
                    wA[p][j] = mask_mult(2 * p + j, 0, wA[p][j])

            sA2 = [score_pair(0, 4 + 2 * p, 5 + 2 * p) for p in range(2)]
            wA2 = [exp_pair(0, 4 + 2 * p, 5 + 2 * p, *sA2[p]) for p in range(2)]
            for pr in range(4):
                v_transpose(pr)
            kts = list(range(8)) + [12, 13, 14, 15, 8, 9, 10, 11]
            wB = {}
            for p in range(4):                # slot B scores kt 0..7
                kt0, kt1 = kts[2 * p], kts[2 * p + 1]
                s0, s1 = score_pair(1, kt0, kt1)
                wB[p] = exp_pair(1, kt0, kt1, s0, s1)

            for p in range(2):                # slot A AV kt 0..3
                for j in range(2):
                    av_accum(avA_e, avA_o, 2 * p + j, wA[p][j],
                             2 * p + j == 0, False)

            kv_block(2)                       # K3 -> kvT 1536:2048
            for p in range(2):                # slot A AV kt 4..7 (covers the
                for j in range(2):            # kv_ps WAR window K3->K2)
                    av_accum(avA_e, avA_o, 4 + 2 * p + j, wA2[p][j],
                             False, 4 + 2 * p + j == 7)
            kv_block(3)                       # K2 -> kvT 1024:1536
            for p in (4, 5):                  # slot B scores kt 12..15
                kt0, kt1 = kts[2 * p], kts[2 * p + 1]
                s0, s1 = score_pair(1, kt0, kt1)
                wB[p] = exp_pair(1, kt0, kt1, s0, s1, defer_mult=True)
            for pr in (6, 7):
                v_transpose(pr)

            for p in (6, 7):                  # slot B scores kt 8..11
                kt0, kt1 = kts[2 * p], kts[2 * p + 1]
                s0, s1 = score_pair(1, kt0, kt1)
                wB[p] = exp_pair(1, kt0, kt1, s0, s1)
            for pr in (4, 5):
                v_transpose(pr)

            # slot A merge on DVE (ACT mid-exp); frees av pool for slot B
            oavA = ep.tile([H + 1, 512], DT, tag="oavA", name="oavA")
            ocA = ep.tile([H + 1, 512], F32, tag="ocA", name="ocA")
            for j in range(4):
                js = slice(j * 128, (j + 1) * 128)
                nc.vector.tensor_copy(ocA[:, js], avA_e[:, js])
                nc.vector.tensor_tensor(oavA[:, js], ocA[:, js],
                                        avA_o[:, js], add)
            # deferred slot-B diagonal mask mults (kt 12..15)
            for p in (4, 5):
                for j in range(2):
                    wB[p][j] = mask_mult(kts[2 * p + j], 1, wB[p][j])

            avB_e = avp.tile([H + 1, 512], F32, tag="avE", name="avE")
            avB_o = avp.tile([H + 1, 512], F32, tag="avO", name="avO")
            for p in range(4):                # slot B AV kt 0..7
                for j in range(2):
                    i = 2 * p + j
                    av_accum(avB_e, avB_o, kts[i], wB[p][j], i == 0, False)

            # slot A transpose + normalize + store
            for j in range(4):
                tr_ps = sp.tile([128, H + 1], DT, tag="score", name="otrA")
                nc.tensor.transpose(tr_ps[:], oavA[:, j * 128:(j + 1) * 128],
                                    id16_sb[:])
                r_sb = ep.tile([128, 1], F32, tag="recip", name="recip")
                nc.vector.reciprocal(r_sb[:], tr_ps[:, H:H + 1])
                o_col = j * H
                nc.vector.tensor_scalar_mul(
                    o_all[:, o_col:o_col + H], tr_ps[:, 0:H], r_sb[:])
            nc.sync.dma_start(out[:, 0:4 * H], o_all[:, 0:4 * H])

            for p in (4, 5, 6, 7):            # slot B AV kt 12..15, 8..11
                for j in range(2):
                    i = 2 * p + j
                    av_accum(avB_e, avB_o, kts[i], wB[p][j], False, i == 15)

            # slot B tail: merge on ACT (free after exps) + DVE
            oavB = ep.tile([H + 1, 512], DT, tag="oavB", name="oavB")
            ocB = ep.tile([H + 1, 512], F32, tag="ocB", name="ocB")
            for j in range(4):
                js = slice(j * 128, (j + 1) * 128)
                nc.scalar.activation(ocB[:, js], avB_e[:, js], Copy)
                nc.vector.tensor_tensor(oavB[:, js], ocB[:, js],
                                        avB_o[:, js], add)
                tr_ps = sp.tile([128, H + 1], DT, tag="score", name="otrB")
                nc.tensor.transpose(tr_ps[:], oavB[:, js], id16_sb[:])
                r_sb = ep.tile([128, 1], F32, tag="recip", name="recip")
                nc.vector.reciprocal(r_sb[:], tr_ps[:, H:H + 1])
                o_col = (4 + j) * H
                nc.vector.tensor_scalar_mul(
                    o_all[:, o_col:o_col + H], tr_ps[:, 0:H], r_sb[:])
            nc.scalar.dma_start(out[:, 4 * H:8 * H], o_all[:, 4 * H:8 * H])

            for pool in (avp, sp, pq, pk):
                pool.release()

    nc.compile()
    return nc


def _host_inputs(x, Wq, bq, Wk, bk, Wv, bv):
    """Build the 8 per-core input maps (all SBUF-layout, fp16/f32)."""
    f16 = np.float16
    Wkv = np.concatenate([Wk, Wv], axis=1)          # [D, 128]

    cst16_np = np.zeros((128, C16_N), dtype=f16)
    for d in range(8):
        cst16_np[:, C_WKV + d * 128:C_WKV + (d + 1) * 128] = \
            Wkv[d * 128:(d + 1) * 128, :]
        cst16_np[:, C_WQ + d * H:C_WQ + (d + 1) * H] = \
            Wq[d * 128:(d + 1) * 128, :]
    cst16_np[:, C_IDV:C_IDV + H] = np.concatenate(
        [np.eye(H), np.eye(H)], axis=0)
    cst16_np[0:H + 1, C_ID16:C_ID16 + H + 1] = np.eye(H + 1)
    xoff = {0: C16_N, 1: C16_N + 4096, 2: C16_N + 8192, 3: C16_N + 16384}

    in_maps = []
    for c in range(8):
        b = c // 2
        cA, cB = c % 2, 3 - c % 2
        perm = (cA, 1 - cA, 5 - cB, cB)        # chunk order along k
        xTp = np.concatenate(
            [x[b, p * CH:(p + 1) * CH].T for p in perm], axis=1)  # [D, S]
        xTp = xTp.astype(f16)
        blob_np = np.zeros((128, BLOB_N), dtype=f16)
        blob_np[:, 0:C16_N] = cst16_np
        for bi in range(4):
            kp = KPOS[bi]
            for d in range(8):
                o = xoff[bi] + d * 512
                blob_np[:, o:o + 512] = \
                    xTp[d * 128:(d + 1) * 128, kp:kp + 512]
        # k_global of permuted position p: perm[p//512]*512 + p%512
        pos = np.arange(S)
        kg = np.array(perm)[pos // CH] * CH + pos % CH
        thr_np = np.zeros((128, 2 * NKT), dtype=np.float32)
        p = np.arange(128)
        for slot, ck in enumerate((cA, cB)):
            for kt in range(NKT):
                thr_np[:, slot * NKT + kt] = kg[kt * 128 + p] - ck * CH
        thrb_np = np.zeros((128, 2 * NKT), dtype=np.float32)
        for slot in range(2):
            for kt in range(NKT):
                diag = (slot == 0 and kt < 4) or (slot == 1 and kt >= 12)
                if diag:
                    continue
                col = thr_np[:, slot * NKT + kt]
                if np.all(col <= 0):
                    continue          # fully visible -> bias 0
                thrb_np[:, slot * NKT + kt] = -1e5   # fully masked
        qio = np.arange(CH, dtype=np.float32)[None, :]
        for m in range(8):
            idx = m if m < 4 else NKT + 8 + m
            o = C16_N + 12288 + m * 512
            blob_np[:, o:o + 512] = \
                (qio >= thr_np[:, idx:idx + 1]).astype(f16)
        cst32_np = np.zeros((128, C32_N), dtype=np.float32)
        cst32_np[:, C_BKV] = np.concatenate([bk, bv])
        cst32_np[0:H, C_BQ] = bq
        cst32_np[:, C_THRB:C_THRB + 2 * NKT] = thrb_np
        in_maps.append({"blob": blob_np, "cst32": cst32_np})
    return in_maps


def _gather(results, dtype):
    y = np.zeros((B, S, H), dtype=dtype)
    for c in range(8):
        b = c // 2
        cA, cB = c % 2, 3 - c % 2
        o = results[c]["out"]
        for slot, ck in enumerate((cA, cB)):
            for j in range(4):
                col = (slot * 4 + j) * H
                y[b, ck * CH + j * 128:ck * CH + (j + 1) * 128] = \
                    o[:, col:col + H]
    return y


def get_nc():
    if "nc" not in _CACHE:
        _CACHE["nc"] = _build_nc()
    return _CACHE["nc"]


def kernel(x, Wq, bq, Wk, bk, Wv, bv, _trace=False, _trace_kwargs=None):
    from concourse.bass_utils import run_bass_kernel_spmd

    x = np.asarray(x, dtype=np.float32)
    Wq, bq = np.asarray(Wq, np.float32), np.asarray(bq, np.float32)
    Wk, bk = np.asarray(Wk, np.float32), np.asarray(bk, np.float32)
    Wv, bv = np.asarray(Wv, np.float32), np.asarray(bv, np.float32)

    nc = get_nc()
    in_maps = _host_inputs(x, Wq, bq, Wk, bk, Wv, bv)
    res = run_bass_kernel_spmd(
        nc, in_maps, core_ids=list(range(8)),
        trace=_trace, **(_trace_kwargs or {}))
    _CACHE["last_result"] = res
    return _gather(res.results, x.dtype)


# revision 25
# speedup vs baseline: 1.2706x; 1.0468x over previous
"""Trainium2 Bass kernel: single-head causal attention, SPMD over 8 NeuronCores.

Problem: x [4, 2048, 1024] f32; Wq/Wk/Wv [1024, 64]; bq/bk/bv [64].
  q,k,v = x@W + b ; out = softmax(causal(q k^T / 8)) @ v  -> [4, 2048, 64]

Sharding (uniform SPMD structure on every core):
  core c -> batch b = c//2 ; query chunks (cA, cB) = (c%2, 3-c%2), 512 rows
  each (pairing an early with a late chunk balances causal work).  Every core
  computes K/V for its batch's full 2048 rows.

Key layout trick: the k-axis is permuted PER CORE to chunk order
  [cA, 1-cA, 5-cB, cB], so the core's own query columns sit at STATIC
  positions 0:512 and 1536:2048 of the K/V input -- Q projection needs no
  separate input tensor.  Causality comes from host-precomputed per-core 0/1
  mask tiles (diagonal tiles) and exp-bias kills (fully-masked tiles), which
  absorb the permutation.

  Projections produce Q^T/K^T/V^T [64, rows]; scores are computed transposed
  ([k_part, q_free]) so the attention-weight matrix feeds the AV matmul as
  the moving operand; V is re-transposed through 16 small PE transposes; a
  65th "ones" row on the V tiles makes the AV matmul accumulate the softmax
  denominator for free.  Score matmuls run UNPACKED (plain K=64, ~0.25us
  extra PE per pair vs row-packing) -- this removes the partition-64:127
  K^T/Q^T duplicate chains (1 DVE add per kv block + 1 ACT ident + the
  qTd-B add) from the exp-spine-gating paths, a measured net win.

Schedule: the hard resource is the ACT engine (24 exp tiles x ~0.58us,
  1 elem/cycle/lane regardless of dtype).  x streams K-MAJOR -- four
  1MB blocks of k-positions (K0=0:512, K1=512:1024, K3=1536:2048,
  K2=1024:1536), each holding all 8 d-tiles for those positions -- so the
  first K/V block (and the first scores-exp) is ready after ~1MB of
  stream + 16 matmuls instead of after the whole 2.5MB h0 half.  The exp
  chain then runs near-gapless while the remaining blocks stream/project.
  Inputs are FEW LARGE DMAs (one dma_start spans all 16 SDMA engines at
  ~340GB/s; issue costs ~0.65us sequencer time each, so count is key) in
  priority order on the sync queue.  Slot-A attention interleaves with the
  later projections on the PE (PSUM: proj kv 1 + q 2 + score 3 + av 2 = 8
  banks; the 2-bank av pool is reused A->B).  Slot-A merge runs on DVE
  (ACT is mid-exp); slot-B merge on ACT (free after the last exp).

dtypes: fp16 SBUF operands, fp32 PSUM accumulation, fp32 normalize.
"""

import os
import sys

import numpy as np

if "/opt/trn_rl_repo" not in sys.path:
    sys.path.insert(0, "/opt/trn_rl_repo")

B, S, D, H = 4, 2048, 1024, 64
CH = 512          # query chunk width
QR = 2 * CH       # query rows per core
NKT = S // 128    # 16 k-tiles of 128
SCALE = 1.0 / np.sqrt(H)

# fp16 blob column layout -- weights/identities at the head
C_WKV = 0          # [128, 1024] 8 d blocks of [128, 128]
C_WQ = 1024        # [128, 512]  8 d blocks of [128, 64]
C_IDV = 1536       # [128, 64]   eye(64) stacked twice
C_ID16 = 1600      # [65, 65]    eye(65)
C16_N = 1696       # padded total
# blob chunk map (name, col0, col1): DMA/stream order
BLOB_CH = [("hd", 0, C16_N + 2048),               # cst16 + K0 d0..3
           ("k0b", C16_N + 2048, C16_N + 4096),   # K0 d4..7
           ("k1", C16_N + 4096, C16_N + 8192),
           ("k3", C16_N + 8192, C16_N + 12288),
           ("msk", C16_N + 12288, C16_N + 16384),
           ("k2", C16_N + 16384, C16_N + 20480)]
BLOB_N = C16_N + 20480

# cst32 column layout (f32)
C_BKV = 0          # [128, 1]
C_BQ = 1           # [64, 1]
C_THRB = 2         # [128, 32] exp bias: 0 or -1e5 (fully-masked kill)
C32_N = 34

# xk k-major block layout: block bi covers k-positions KPOS[bi]:+512 with
# 8 d-tiles of [128, 512] each; stream order K0, K1, K3, K2.
KPOS = (0, 512, 1536, 1024)
# kvT 512-col block index (nb) per stream block
KNB = (0, 1, 3, 2)

_CACHE = {}


def _build_nc():
    import concourse.bacc as bacc
    import concourse.mybir as mybir
    import concourse.tile as tile

    DT = mybir.dt.float16
    F32 = mybir.dt.float32
    Exp = mybir.ActivationFunctionType.Exp
    Copy = mybir.ActivationFunctionType.Copy
    Ident = mybir.ActivationFunctionType.Identity
    mult = mybir.AluOpType.mult
    add = mybir.AluOpType.add

    nc = bacc.Bacc("TRN2", target_bir_lowering=False, debug=False, num_devices=8)

    blob = nc.dram_tensor("blob", [128, BLOB_N], DT, kind="ExternalInput")
    cst32 = nc.dram_tensor("cst32", [128, C32_N], F32, kind="ExternalInput")
    out = nc.dram_tensor("out", [128, 8 * H], DT, kind="ExternalOutput")

    with tile.TileContext(nc) as tc:
        with (
            tc.tile_pool(name="const", bufs=1) as cp,
            tc.tile_pool(name="work", bufs=16) as wp,
            tc.tile_pool(name="epi", bufs=4) as ep,
        ):
            # ---- head: 6 large DMAs in priority order on the sync
            # queue; weights+K0a share ONE DMA (each extra dma_start on the
            # critical path costs ~1us issue+drain-start lag)
            tiles = {}
            for nm, c0, c1 in BLOB_CH:
                tiles[nm] = cp.tile([128, c1 - c0], DT, tag=nm, name=nm)
                nc.sync.dma_start(tiles[nm][:], blob[:, c0:c1])
            hd_sb, xb0b = tiles["hd"], tiles["k0b"]
            xb1, xb3, xb2 = tiles["k1"], tiles["k3"], tiles["k2"]
            msk_sb = tiles["msk"]
            cst32_sb = cp.tile([128, C32_N], F32, tag="cst32", name="cst32")
            nc.gpsimd.dma_start(cst32_sb[:], cst32[:])

            wkv_sb = hd_sb[:, C_WKV:C_WKV + 1024]
            wq_sb = hd_sb[:, C_WQ:C_WQ + 512]
            idv_sb = hd_sb[:, C_IDV:C_IDV + H]
            id16_sb = hd_sb[0:H + 1, C_ID16:C_ID16 + H + 1]
            bkv_sb = cst32_sb[:, C_BKV:C_BKV + 1]
            bq_sb = cst32_sb[0:H, C_BQ:C_BQ + 1]
            thrb_sb = cst32_sb[:, C_THRB:C_THRB + 2 * NKT]

            def xs(bi, d):    # stream block bi, d-tile d -> [128, 512]
                if bi == 0:
                    if d < 4:
                        return hd_sb[:, C16_N + d * 512:C16_N + (d + 1) * 512]
                    return xb0b[:, (d - 4) * 512:(d - 3) * 512]
                t = (xb1, xb3, xb2)[bi - 1]
                return t[:, d * 512:(d + 1) * 512]

            kvT_sb = cp.tile([128, S], DT, tag="kvT", name="kvT")  # 0:64 K^T, 64:128 V^T
            qT_sb = cp.tile([H, QR], DT, tag="qT", name="qT")      # A cols 0:512, B 512:1024
            v_sb = cp.tile([128, NKT * (H + 1)], DT, tag="v", name="v")
            vtd_sb = cp.tile([64, S], DT, tag="vtd", name="vtd")
            o_all = cp.tile([128, 8 * H], DT, tag="oall", name="oall")

            nc.vector.memset(v_sb[:], 1.0)

            pk = tc.alloc_tile_pool(name="proj_ps", bufs=1, space="PSUM")
            pq = tc.alloc_tile_pool(name="q_ps", bufs=1, space="PSUM")
            sp = tc.alloc_tile_pool(name="score_ps", bufs=4, space="PSUM")
            avp = tc.alloc_tile_pool(name="av_ps", bufs=1, space="PSUM")

            def kv_block(bi):
                """8 accumulating matmuls for one 512-position k block, then
                DVE epilogue (ktd dup first: it unblocks row-packed scores)."""
                kv_ps = pk.tile([128, 512], F32, tag="kvps", name="kvps")
                for d in range(8):
                    nc.tensor.matmul(
                        kv_ps[:], wkv_sb[:, d * 128:(d + 1) * 128],
                        xs(bi, d), start=(d == 0), stop=(d == 7))
                nb = KNB[bi]
                cs = slice(nb * 512, (nb + 1) * 512)
                nc.vector.tensor_scalar(kvT_sb[:, cs], kv_ps[:],
                                        bkv_sb[:], None, add)
                nc.vector.tensor_scalar(vtd_sb[:, cs], kv_ps[H:128, :],
                                        bkv_sb[H:128, :], None, add)

            def q_block(bi, slot):
                q_ps = pq.tile([H, 512], F32, tag="qps", name="qps")
                for d in range(8):
                    nc.tensor.matmul(
                        q_ps[:], wq_sb[:, d * H:(d + 1) * H],
                        xs(bi, d), start=(d == 0), stop=(d == 7))
                cs = slice(slot * 512, (slot + 1) * 512)
                if slot == 0:     # ACT is idle pre-exp; keep DVE for kv epis
                    nc.scalar.activation(qT_sb[:, cs], q_ps[:], Ident,
                                         bias=bq_sb[:])
                else:             # ACT is mid-exp by now
                    nc.vector.tensor_scalar(qT_sb[:, cs], q_ps[:],
                                            bq_sb[:], None, add)

            def v_transpose(pr):
                k0, k1 = 2 * pr, 2 * pr + 1
                t0 = sp.tile([128, H], DT, tag="score", name="vtr0")
                nc.tensor.transpose(
                    t0[:], vtd_sb[:, k0 * 128:(k0 + 1) * 128],
                    idv_sb[0:H, :], tile_position=(0, 0))
                t1 = sp.tile([128, H], DT, tag="score", name="vtr1")
                nc.tensor.transpose(
                    t1[:], kvT_sb[64:128, k1 * 128:(k1 + 1) * 128],
                    idv_sb[64:64 + H, :], tile_position=(64, 0))
                nc.vector.tensor_copy(
                    v_sb[:, k0 * (H + 1):k0 * (H + 1) + H], t0[:])
                nc.vector.tensor_copy(
                    v_sb[:, k1 * (H + 1):k1 * (H + 1) + H], t1[:])

            def score_pair(slot, kt0, kt1):
                s0 = sp.tile([128, 512], F32, tag="score", name="score0")
                nc.tensor.matmul(
                    s0[:], kvT_sb[0:H, kt0 * 128:(kt0 + 1) * 128],
                    qT_sb[:, slot * 512:(slot + 1) * 512],
                    start=True, stop=True)
                s1 = sp.tile([128, 512], F32, tag="score", name="score1")
                nc.tensor.matmul(
                    s1[:], kvT_sb[0:H, kt1 * 128:(kt1 + 1) * 128],
                    qT_sb[:, slot * 512:(slot + 1) * 512],
                    start=True, stop=True)
                return s0, s1

            def mask_mult(kt, slot, w_sb):
                m = kt if slot == 0 else kt - 8
                wm_sb = wp.tile([128, 512], DT, tag="wm", name="wm")
                nc.vector.tensor_tensor(
                    wm_sb[:], w_sb[:], msk_sb[:, m * 512:(m + 1) * 512], mult)
                return wm_sb

            def exp_pair(slot, kt0, kt1, s0, s1, defer_mult=False):
                """exps on ACT; diag tiles get a DVE mask multiply, which can
                be deferred so exp-gated mults don't clog the DVE queue
                ahead of spine-critical kv epilogues."""
                res = []
                for kt, s_ps in zip((kt0, kt1), (s0, s1)):
                    idx = slot * NKT + kt
                    w_sb = wp.tile([128, 512], DT, tag="wexp", name="wexp")
                    nc.scalar.activation(w_sb[:], s_ps[:], Exp,
                                         bias=thrb_sb[:, idx:idx + 1],
                                         scale=float(SCALE))
                    diag = (slot == 0 and kt < 4) or (slot == 1 and kt >= 12)
                    if diag and not defer_mult:
                        res.append(mask_mult(kt, slot, w_sb))
                    else:
                        res.append(w_sb)
                return res

            def av_accum(av_e, av_o, kt, w_av, first, last):
                vs = slice(kt * (H + 1), (kt + 1) * (H + 1))
                nc.tensor.matmul(
                    av_e[:], v_sb[0:H, vs], w_av[0:H, :],
                    start=first, stop=last, tile_position=(0, 0))
                nc.tensor.matmul(
                    av_o[:], v_sb[H:128, vs], w_av[H:128, :],
                    start=first, stop=last, tile_position=(64, 0))

            # ================= emission (PE-queue order) =================
            kv_block(0)                       # K0 -> kvT 0:512
            q_block(0, 0)                     # qA (+ ACT idents)
            kv_block(1)                       # K1 -> kvT 512:1024
            # (kv1's DVE epilogue queues BEFORE slot-A mults/v-copies so
            # the kt4..7 scores are never DVE-starved)

            avA_e = avp.tile([H + 1, 512], F32, tag="avE", name="avE")
            avA_o = avp.tile([H + 1, 512], F32, tag="avO", name="avO")
            sA = [score_pair(0, 2 * p, 2 * p + 1) for p in range(2)]
            wA = [exp_pair(0, 2 * p, 2 * p + 1, *sA[p], defer_mult=True)
                  for p in range(2)]

            q_block(2, 1)                     # qB early: its DVE epilogue
            # (qTd-B) gates the slot-B half of the exp spine
            for p in range(2):
                for j in range(2):
                    wA[p][j] = mask_mult(2 * p + j, 0, wA[p][j])

            sA2 = [score_pair(0, 4 + 2 * p, 5 + 2 * p) for p in range(2)]
            wA2 = [exp_pair(0, 4 + 2 * p, 5 + 2 * p, *sA2[p]) for p in range(2)]
            for pr in range(4):
                v_transpose(pr)
            kts = list(range(8)) + [12, 13, 14, 15, 8, 9, 10, 11]
            wB = {}
            for p in range(4):                # slot B scores kt 0..7
                kt0, kt1 = kts[2 * p], kts[2 * p + 1]
                s0, s1 = score_pair(1, kt0, kt1)
                wB[p] = exp_pair(1, kt0, kt1, s0, s1)

            for p in range(2):                # slot A AV kt 0..3
                for j in range(2):
                    av_accum(avA_e, avA_o, 2 * p + j, wA[p][j],
                             2 * p + j == 0, False)

            kv_block(2)                       # K3 -> kvT 1536:2048
            for p in range(2):                # slot A AV kt 4..7 (covers the
                for j in range(2):            # kv_ps WAR window K3->K2)
                    av_accum(avA_e, avA_o, 4 + 2 * p + j, wA2[p][j],
                             False, 4 + 2 * p + j == 7)
            kv_block(3)                       # K2 -> kvT 1024:1536
            for p in (4, 5):                  # slot B scores kt 12..15
                kt0, kt1 = kts[2 * p], kts[2 * p + 1]
                s0, s1 = score_pair(1, kt0, kt1)
                wB[p] = exp_pair(1, kt0, kt1, s0, s1, defer_mult=True)
            for pr in (6, 7):
                v_transpose(pr)

            for p in (6, 7):                  # slot B scores kt 8..11
                kt0, kt1 = kts[2 * p], kts[2 * p + 1]
                s0, s1 = score_pair(1, kt0, kt1)
                wB[p] = exp_pair(1, kt0, kt1, s0, s1)
            for pr in (4, 5):
                v_transpose(pr)

            # slot A merge on DVE (ACT mid-exp); frees av pool for slot B
            oavA = ep.tile([H + 1, 512], DT, tag="oavA", name="oavA")
            ocA = ep.tile([H + 1, 512], F32, tag="ocA", name="ocA")
            for j in range(4):
                js = slice(j * 128, (j + 1) * 128)
                nc.vector.tensor_copy(ocA[:, js], avA_e[:, js])
                nc.vector.tensor_tensor(oavA[:, js], ocA[:, js],
                                        avA_o[:, js], add)
            # deferred slot-B diagonal mask mults (kt 12..15)
            for p in (4, 5):
                for j in range(2):
                    wB[p][j] = mask_mult(kts[2 * p + j], 1, wB[p][j])

            avB_e = avp.tile([H + 1, 512], F32, tag="avE", name="avE")
            avB_o = avp.tile([H + 1, 512], F32, tag="avO", name="avO")
            for p in range(4):                # slot B AV kt 0..7
                for j in range(2):
                    i = 2 * p + j
                    av_accum(avB_e, avB_o, kts[i], wB[p][j], i == 0, False)

            # slot A transpose + normalize + store
            for j in range(4):
                tr_ps = sp.tile([128, H + 1], DT, tag="score", name="otrA")
                nc.tensor.transpose(tr_ps[:], oavA[:, j * 128:(j + 1) * 128],
                                    id16_sb[:])
                r_sb = ep.tile([128, 1], F32, tag="recip", name="recip")
                nc.vector.reciprocal(r_sb[:], tr_ps[:, H:H + 1])
                o_col = j * H
                nc.vector.tensor_scalar_mul(
                    o_all[:, o_col:o_col + H], tr_ps[:, 0:H], r_sb[:])
            nc.sync.dma_start(out[:, 0:4 * H], o_all[:, 0:4 * H])

            for p in (4, 5, 6, 7):            # slot B AV kt 12..15, 8..11
                for j in range(2):
                    i = 2 * p + j
                    av_accum(avB_e, avB_o, kts[i], wB[p][j], False, i == 15)

            # slot B tail: merge on ACT (free after exps) + DVE
            oavB = ep.tile([H + 1, 512], DT, tag="oavB", name="oavB")
            ocB = ep.tile([H + 1, 512], F32, tag="ocB", name="ocB")
            for j in range(4):
                js = slice(j * 128, (j + 1) * 128)
                nc.scalar.activation(ocB[:, js], avB_e[:, js], Copy)
                nc.vector.tensor_tensor(oavB[:, js], ocB[:, js],
                                        avB_o[:, js], add)
                tr_ps = sp.tile([128, H + 1], DT, tag="score", name="otrB")
                nc.tensor.transpose(tr_ps[:], oavB[:, js], id16_sb[:])
                r_sb = ep.tile([128, 1], F32, tag="recip", name="recip")
                nc.vector.reciprocal(r_sb[:], tr_ps[:, H:H + 1])
                o_col = (4 + j) * H
                nc.vector.tensor_scalar_mul(
                    o_all[:, o_col:o_col + H], tr_ps[:, 0:H], r_sb[:])
            nc.scalar.dma_start(out[:, 4 * H:8 * H], o_all[:, 4 * H:8 * H])

            for pool in (avp, sp, pq, pk):
                pool.release()

    nc.compile()
    return nc


def _host_inputs(x, Wq, bq, Wk, bk, Wv, bv):
    """Build the 8 per-core input maps (all SBUF-layout, fp16/f32)."""
    f16 = np.float16
    Wkv = np.concatenate([Wk, Wv], axis=1)          # [D, 128]

    cst16_np = np.zeros((128, C16_N), dtype=f16)
    for d in range(8):
        cst16_np[:, C_WKV + d * 128:C_WKV + (d + 1) * 128] = \
            Wkv[d * 128:(d + 1) * 128, :]
        cst16_np[:, C_WQ + d * H:C_WQ + (d + 1) * H] = \
            Wq[d * 128:(d + 1) * 128, :]
    cst16_np[:, C_IDV:C_IDV + H] = np.concatenate(
        [np.eye(H), np.eye(H)], axis=0)
    cst16_np[0:H + 1, C_ID16:C_ID16 + H + 1] = np.eye(H + 1)
    xoff = {0: C16_N, 1: C16_N + 4096, 2: C16_N + 8192, 3: C16_N + 16384}

    in_maps = []
    for c in range(8):
        b = c // 2
        cA, cB = c % 2, 3 - c % 2
        perm = (cA, 1 - cA, 5 - cB, cB)        # chunk order along k
        xTp = np.concatenate(
            [x[b, p * CH:(p + 1) * CH].T for p in perm], axis=1)  # [D, S]
        xTp = xTp.astype(f16)
        blob_np = np.zeros((128, BLOB_N), dtype=f16)
        blob_np[:, 0:C16_N] = cst16_np
        for bi in range(4):
            kp = KPOS[bi]
            for d in range(8):
                o = xoff[bi] + d * 512
                blob_np[:, o:o + 512] = \
                    xTp[d * 128:(d + 1) * 128, kp:kp + 512]
        # k_global of permuted position p: perm[p//512]*512 + p%512
        pos = np.arange(S)
        kg = np.array(perm)[pos // CH] * CH + pos % CH
        thr_np = np.zeros((128, 2 * NKT), dtype=np.float32)
        p = np.arange(128)
        for slot, ck in enumerate((cA, cB)):
            for kt in range(NKT):
                thr_np[:, slot * NKT + kt] = kg[kt * 128 + p] - ck * CH
        thrb_np = np.zeros((128, 2 * NKT), dtype=np.float32)
        for slot in range(2):
            for kt in range(NKT):
                diag = (slot == 0 and kt < 4) or (slot == 1 and kt >= 12)
                if diag:
                    continue
                col = thr_np[:, slot * NKT + kt]
                if np.all(col <= 0):
                    continue          # fully visible -> bias 0
                thrb_np[:, slot * NKT + kt] = -1e5   # fully masked
        qio = np.arange(CH, dtype=np.float32)[None, :]
        for m in range(8):
            idx = m if m < 4 else NKT + 8 + m
            o = C16_N + 12288 + m * 512
            blob_np[:, o:o + 512] = \
                (qio >= thr_np[:, idx:idx + 1]).astype(f16)
        cst32_np = np.zeros((128, C32_N), dtype=np.float32)
        cst32_np[:, C_BKV] = np.concatenate([bk, bv])
        cst32_np[0:H, C_BQ] = bq
        cst32_np[:, C_THRB:C_THRB + 2 * NKT] = thrb_np
        in_maps.append({"blob": blob_np, "cst32": cst32_np})
    return in_maps


def _gather(results, dtype):
    y = np.zeros((B, S, H), dtype=dtype)
    for c in range(8):
        b = c // 2
        cA, cB = c % 2, 3 - c % 2
        o = results[c]["out"]
        for slot, ck in enumerate((cA, cB)):
            for j in range(4):
                col = (slot * 4 + j) * H
                y[b, ck * CH + j * 128:ck * CH + (j + 1) * 128] = \
                    o[:, col:col + H]
    return y


def get_nc():
    if "nc" not in _CACHE:
        _CACHE["nc"] = _build_nc()
    return _CACHE["nc"]


def kernel(x, Wq, bq, Wk, bk, Wv, bv, _trace=False, _trace_kwargs=None):
    from concourse.bass_utils import run_bass_kernel_spmd

    x = np.asarray(x, dtype=np.float32)
    Wq, bq = np.asarray(Wq, np.float32), np.asarray(bq, np.float32)
    Wk, bk = np.asarray(Wk, np.float32), np.asarray(bk, np.float32)
    Wv, bv = np.asarray(Wv, np.float32), np.asarray(bv, np.float32)

    nc = get_nc()
    in_maps = _host_inputs(x, Wq, bq, Wk, bk, Wv, bv)
    res = run_bass_kernel_spmd(
        nc, in_maps, core_ids=list(range(8)),
        trace=_trace, **(_trace_kwargs or {}))
    _CACHE["last_result"] = res
    return _gather(res.results, x.dtype)
